# revision 11
# baseline (speedup 1.0000x reference)
"""Trainium2 Bass kernel for nn_KPFBNC (pattern-similarity graph + losses).

Computes, per batch sample b:
  graphs[b] = kp^T diag(PES[b]) kp * non_diag_scale
plus scalar losses (norm, orth) from the patterns alone.

Sharding: pure data parallelism over batch B=64 across 8 NeuronCores
(8 samples per core); patterns replicated; losses computed redundantly
on every core (taken from core 0).

Device pipeline per core:
  prep:   normalize patterns, top-3 mask, losses, PE-transpose -> patT
  per sample:
    xsq = x*x (DVE) -> column sums via ones-matmul (PE) ->
    invf = exp(-0.5*ln(fnorm2+eps)) (ACT, 4-sample batched rows) ->
    DMA broadcast of invf -> x_norm = x*invf (DVE) ->
    sim = patT^T @ x_norm (PE, fp16 in / f32 accum) ->
    ACT drains PSUM->SBUF fp16 -> windowed max/min reduces (DVE) ->
    PES -> graphs matmuls (PE) -> *non_diag_scale (DVE) -> DMA out
"""

import numpy as np
import ml_dtypes  # noqa: F401  (np float16 used; bf16 avail if needed)

import concourse.bass as bass
import concourse.bacc as bacc
import concourse.tile as tile
from concourse import mybir
from concourse.bass_utils import run_bass_kernel_spmd

# ---- problem constants (hardcoded per contract) ----
B, N, P, T = 64, 200, 400, 1200
TOPK, STRIDE = 3, 25
THRESHOLD = 0.3
EPS = 1e-9
NCORES = 8
BL = B // NCORES            # samples per core
W = T // STRIDE             # 48 windows
C_SCALE = TOPK * P / N      # 6.0 non-diag scale

F32 = mybir.dt.float32
F16 = mybir.dt.float16
AX = mybir.AxisListType
OP = mybir.AluOpType
AF = mybir.ActivationFunctionType

P_TILES = [(0, 128), (128, 128), (256, 128), (384, 16)]
N_TILES = [(0, 128), (128, 72)]
T_CHUNKS = [(0, 512), (512, 512), (1024, 176)]

BIG = 1e30


def _emit(nc, tc, ctx):
    # ---------------- IO ----------------
    x_t = nc.dram_tensor("x16", [BL, N, T], F16, kind="ExternalInput")
    pat_t = nc.dram_tensor("patterns", [P, N], F32, kind="ExternalInput")
    isap_t = nc.dram_tensor("invsap2", [128, BL], F32, kind="ExternalInput")
    g_t = nc.dram_tensor("graphs", [BL, N, N], F32, kind="ExternalOutput")
    norm_t = nc.dram_tensor("norm", [1, 1], F32, kind="ExternalOutput")
    orth_t = nc.dram_tensor("orth", [1, 1], F32, kind="ExternalOutput")
    invf_scr = nc.dram_tensor("invf_scr", [BL, T], F16, kind="Internal")

    # ---------------- inline consts ----------------
    ident_c = nc.inline_tensor(np.eye(128, dtype=np.float32), "identc")
    ones32_c = nc.inline_tensor(np.ones((128, 1), np.float32), "ones32c")
    ones16_c = nc.inline_tensor(np.ones((128, 1), np.float16), "ones16c")
    dmask_np = np.full((N, N), C_SCALE, np.float32) - np.eye(N, dtype=np.float32) * (
        C_SCALE - 1.0
    )
    dmask_c = nc.inline_tensor(dmask_np, "dmaskc")
    eyec_np = np.zeros((128, 16), np.float16)
    for _b in range(4):
        eyec_np[:, 4 * _b + _b] = 1.0
    eyec_c = nc.inline_tensor(eyec_np, "eyecolsc")

    statics = ctx.enter_context(tc.tile_pool(name="statics", bufs=1))

    ident = statics.tile([128, 128], F32, tag="ident")
    nc.sync.dma_start(out=ident, in_=ident_c.ap())
    ones32 = statics.tile([128, 1], F32, tag="ones32")
    nc.sync.dma_start(out=ones32, in_=ones32_c.ap())
    ones16 = statics.tile([128, 1], F16, tag="ones16")
    nc.sync.dma_start(out=ones16, in_=ones16_c.ap())
    eyecols = statics.tile([128, 16], F16, tag="eyecols")
    nc.sync.dma_start(out=eyecols, in_=eyec_c.ap())
    dmask = []
    for mi, (n0, nsz) in enumerate(N_TILES):
        dm = statics.tile([nsz, N], F32, tag=f"dmask{mi}")
        nc.sync.dma_start(out=dm, in_=dmask_c.ap()[n0 : n0 + nsz, :])
        dmask.append(dm)
    isap = statics.tile([128, BL], F32, tag="isap")
    nc.sync.dma_start(out=isap, in_=isap_t.ap())
    epsc = statics.tile([128, 1], F32, tag="epsc")
    nc.vector.memset(epsc, EPS)

    # =========================================================
    # PREP: patterns -> npat, top-3, losses, patT (f16 + f32)
    # =========================================================
    prep_cm = tc.tile_pool(name="prep", bufs=1)
    prep_ps_cm = tc.tile_pool(name="prep_ps", bufs=1, space="PSUM")
    loss_ps_cm = tc.tile_pool(name="loss_ps", bufs=1, space="PSUM")
    prep = prep_cm.__enter__()
    prep_ps = prep_ps_cm.__enter__()
    loss_ps = loss_ps_cm.__enter__()

    npat = []      # normalized patterns f32, per P tile
    top = []       # toppattern f32
    kp16 = []      # toppattern f16
    lenp2 = []     # per-row sum npat^2

    ps_norm = loss_ps.tile([1, 1], F32, tag="ps_norm")
    ps_orth = loss_ps.tile([1, 1], F32, tag="ps_orth")

    for pi, (p0, psz) in enumerate(P_TILES):
        pat = prep.tile([psz, N], F32, tag=f"pat{pi}")
        nc.sync.dma_start(out=pat, in_=pat_t.ap()[p0 : p0 + psz, :])

        scr = prep.tile([psz, N], F32, tag=f"scr{pi}")
        pn2 = prep.tile([psz, 1], F32, tag=f"pn2{pi}")
        nc.vector.scalar_tensor_tensor(
            out=scr, in0=pat, scalar=1.0, in1=pat, op0=OP.bypass, op1=OP.mult,
            accum_out=pn2,
        )
        slen = prep.tile([psz, 1], F32, tag=f"slen{pi}")
        nc.scalar.activation(out=slen, in_=pn2, func=AF.Sqrt,
                             bias=epsc[0:psz, 0:1], scale=1.0)
        invl = prep.tile([psz, 1], F32, tag=f"invl{pi}")
        nc.vector.reciprocal(out=invl, in_=slen)
        np_i = prep.tile([psz, N], F32, tag=f"npat{pi}")
        nc.vector.tensor_scalar_mul(out=np_i, in0=pat, scalar1=invl[:, 0:1])
        npat.append(np_i)

        # ---- top-3 by |npat| ----
        a = prep.tile([psz, N], F32, tag=f"abs{pi}")
        nc.scalar.activation(out=a, in_=np_i, func=AF.Abs)
        m1 = prep.tile([psz, 1], F32, tag=f"m1{pi}")
        nc.vector.reduce_max(out=m1, in_=a, axis=AX.X)
        e = prep.tile([psz, N], F32, tag=f"e{pi}")
        nc.vector.tensor_scalar(out=e, in0=a, scalar1=m1[:, 0:1], scalar2=None,
                                op0=OP.is_equal)
        a1 = prep.tile([psz, N], F32, tag=f"a1{pi}")
        nc.vector.scalar_tensor_tensor(
            out=a1, in0=e, scalar=-BIG, in1=a, op0=OP.mult, op1=OP.add)
        m2 = prep.tile([psz, 1], F32, tag=f"m2{pi}")
        nc.vector.reduce_max(out=m2, in_=a1, axis=AX.X)
        nc.vector.tensor_scalar(out=e, in0=a1, scalar1=m2[:, 0:1], scalar2=None,
                                op0=OP.is_equal)
        a2 = prep.tile([psz, N], F32, tag=f"a2{pi}")
        nc.vector.scalar_tensor_tensor(
            out=a2, in0=e, scalar=-BIG, in1=a1, op0=OP.mult, op1=OP.add)
        m3 = prep.tile([psz, 1], F32, tag=f"m3{pi}")
        nc.vector.reduce_max(out=m3, in_=a2, axis=AX.X)
        mask = prep.tile([psz, N], F32, tag=f"mask{pi}")
        nc.vector.tensor_scalar(out=mask, in0=a, scalar1=m3[:, 0:1], scalar2=None,
                                op0=OP.is_ge)
        t_i = statics.tile([psz, N], F32, tag=f"top{pi}")
        nc.vector.tensor_mul(t_i, mask, np_i)
        top.append(t_i)
        k_i = statics.tile([psz, N], F16, tag=f"kp16{pi}")
        nc.vector.tensor_copy(out=k_i, in_=t_i)
        kp16.append(k_i)

        # ---- norm loss pieces ----
        l2 = prep.tile([psz, 1], F32, tag=f"lenp2{pi}")
        nc.vector.scalar_tensor_tensor(
            out=scr, in0=np_i, scalar=1.0, in1=np_i, op0=OP.bypass, op1=OP.mult,
            accum_out=l2)
        lenp2.append(l2)
        q = prep.tile([psz, 1], F32, tag=f"q{pi}")
        nc.vector.scalar_tensor_tensor(
            out=scr, in0=t_i, scalar=1.0, in1=t_i, op0=OP.bypass, op1=OP.mult,
            accum_out=q)
        il2 = prep.tile([psz, 1], F32, tag=f"il2{pi}")
        nc.vector.reciprocal(out=il2, in_=l2)
        r2 = prep.tile([psz, 1], F32, tag=f"r2{pi}")
        nc.vector.tensor_mul(r2, q, il2)
        r = prep.tile([psz, 1], F32, tag=f"r{pi}")
        nc.scalar.activation(out=r, in_=r2, func=AF.Sqrt)
        tm = prep.tile([psz, 1], F32, tag=f"tm{pi}")
        nc.vector.tensor_scalar(out=tm, in0=r, scalar1=-1.0, scalar2=1.0,
                                op0=OP.mult, op1=OP.add)
        tm2 = prep.tile([psz, 1], F32, tag=f"tm2{pi}")
        nc.vector.tensor_mul(tm2, tm, tm)
        nc.tensor.matmul(ps_norm[0:1, 0:1], lhsT=tm2[:, 0:1], rhs=ones32[0:psz, 0:1],
                         start=(pi == 0), stop=(pi == len(P_TILES) - 1),
                         skip_group_check=True)

    # ---- transpose npat -> patT (two N-row tiles of (nsz, P)) ----
    patT16 = []
    patT32 = []
    for ni, (n0, nsz) in enumerate(N_TILES):
        ps_pt = prep_ps.tile([nsz, P], F32, tag=f"ps_patT{ni}")
        for pi, (p0, psz) in enumerate(P_TILES):
            nc.tensor.transpose(
                ps_pt[0:nsz, p0 : p0 + psz],
                npat[pi][:, n0 : n0 + nsz],
                ident[0:psz, 0:psz],
            )
        pt32 = statics.tile([nsz, P], F32, tag=f"patT32_{ni}")
        nc.scalar.copy(out=pt32, in_=ps_pt)
        pt16 = statics.tile([nsz, P], F16, tag=f"patT16_{ni}")
        nc.vector.tensor_copy(out=pt16, in_=ps_pt)
        patT16.append(pt16)
        patT32.append(pt32)

    # ---- orth loss ----
    for pi, (p0, psz) in enumerate(P_TILES):
        ps_o = prep_ps.tile([psz, P], F32, tag="ps_simpp")
        for ni, (n0, nsz) in enumerate(N_TILES):
            nc.tensor.matmul(ps_o, lhsT=patT32[ni][:, p0 : p0 + psz],
                             rhs=patT32[ni][:, :],
                             start=(ni == 0), stop=(ni == 1))
        ab = prep.tile([psz, P], F32, tag=f"oab{pi}")
        nc.scalar.activation(out=ab, in_=ps_o, func=AF.Abs)
        z = prep.tile([psz, P], F32, tag=f"oz{pi}")
        nc.vector.tensor_scalar(out=z, in0=ab, scalar1=-THRESHOLD,
                                scalar2=1.0 / (1.0 - THRESHOLD + 1e-5),
                                op0=OP.add, op1=OP.mult)
        rl = prep.tile([psz, P], F32, tag=f"orl{pi}")
        nc.vector.tensor_scalar_max(out=rl, in0=z, scalar1=0.0)
        oscr = prep.tile([psz, P], F32, tag=f"oscr{pi}")
        osum = prep.tile([psz, 1], F32, tag=f"osum{pi}")
        nc.vector.scalar_tensor_tensor(
            out=oscr, in0=rl, scalar=1.0, in1=rl, op0=OP.bypass, op1=OP.mult,
            accum_out=osum)
        # subtract the diagonal contribution relu((lenp2 - th)/(1-th+1e-5))^2
        zd = prep.tile([psz, 1], F32, tag=f"ozd{pi}")
        nc.vector.tensor_scalar(out=zd, in0=lenp2[pi], scalar1=-THRESHOLD,
                                scalar2=1.0 / (1.0 - THRESHOLD + 1e-5),
                                op0=OP.add, op1=OP.mult)
        nc.vector.tensor_scalar_max(out=zd, in0=zd, scalar1=0.0)
        d2 = prep.tile([psz, 1], F32, tag=f"od2{pi}")
        nc.vector.tensor_mul(d2, zd, zd)
        nc.vector.tensor_sub(osum, osum, d2)
        nc.tensor.matmul(ps_orth[0:1, 0:1], lhsT=osum[:, 0:1],
                         rhs=ones32[0:psz, 0:1],
                         start=(pi == 0), stop=(pi == len(P_TILES) - 1),
                         skip_group_check=True)

    normv = prep.tile([1, 1], F32, tag="normv")
    nc.scalar.mul(out=normv, in_=ps_norm, mul=1.0 / P)
    nc.sync.dma_start(out=norm_t.ap(), in_=normv)
    orthv = prep.tile([1, 1], F32, tag="orthv")
    nc.scalar.mul(out=orthv, in_=ps_orth, mul=1.0 / (P * P))
    nc.sync.dma_start(out=orth_t.ap(), in_=orthv)

    loss_ps_cm.__exit__(None, None, None)
    prep_ps_cm.__exit__(None, None, None)
    prep_cm.__exit__(None, None, None)

    # =========================================================
    # PER-SAMPLE PIPELINE
    # =========================================================
    xp = ctx.enter_context(tc.tile_pool(name="xp", bufs=5))
    sq = ctx.enter_context(tc.tile_pool(name="sq", bufs=5))
    rows = ctx.enter_context(tc.tile_pool(name="rows", bufs=2))
    work = ctx.enter_context(tc.tile_pool(name="work", bufs=2))
    simp = ctx.enter_context(tc.tile_pool(name="simp", bufs=3))
    fn_ps = ctx.enter_context(tc.tile_pool(name="fn_ps", bufs=1, space="PSUM"))
    sim_ps = ctx.enter_context(tc.tile_pool(name="sim_ps", bufs=2, space="PSUM"))
    g_ps = ctx.enter_context(tc.tile_pool(name="g_ps", bufs=1, space="PSUM"))

    xa = {}
    xb = {}
    xsqa = {}
    xsqb = {}

    def phase1_load(b):
        xa[b] = xp.tile([128, T], F16, tag="xa", name=f"xa{b}")
        nc.sync.dma_start(out=xa[b], in_=x_t.ap()[b, 0:128, :])
        xb[b] = xp.tile([72, T], F16, tag="xb", name=f"xb{b}")
        nc.sync.dma_start(out=xb[b], in_=x_t.ap()[b, 128:200, :])
        xsqa[b] = sq.tile([128, T], F16, tag="xsqa", name=f"xsqa{b}")
        nc.gpsimd.tensor_mul(xsqa[b], xa[b], xa[b])
        xsqb[b] = sq.tile([72, T], F16, tag="xsqb", name=f"xsqb{b}")
        nc.gpsimd.tensor_mul(xsqb[b], xb[b], xb[b])

    def phase2_group(g):
        grp = range(4 * g, 4 * g + 4)
        lnf = rows.tile([4, T], F32, tag="lnf")
        for c, (c0, csz) in enumerate(T_CHUNKS):
            fn = fn_ps.tile([4, 512], F32, tag="fn")
            for k, b in enumerate(grp):
                for ni, (n0, nsz) in enumerate(N_TILES):
                    xs = (xsqa[b] if ni == 0 else xsqb[b])
                    nc.tensor.matmul(
                        fn[0:4, 0:csz],
                        lhsT=eyecols[0:nsz, 4 * k : 4 * k + 4],
                        rhs=xs[:, c0 : c0 + csz],
                        start=(k == 0 and ni == 0), stop=(k == 3 and ni == 1),
                        skip_group_check=True,
                    )
            nc.scalar.activation(
                out=lnf[0:4, c0 : c0 + csz],
                in_=fn[0:4, 0:csz],
                func=AF.Ln, bias=epsc[0:4, 0:1], scale=1.0,
            )
        invf = rows.tile([4, T], F16, tag="invf")
        nc.scalar.activation(out=invf[0:4, :], in_=lnf[0:4, :],
                             func=AF.Exp, scale=-0.5)
        for k, b in enumerate(grp):
            nc.sync.dma_start(out=invf_scr.ap()[b : b + 1, :],
                              in_=invf[k : k + 1, :])

    def phase3_sample(b):
        bc = work.tile([128, T], F16, tag="bc")
        nc.sync.dma_start(out=bc, in_=invf_scr.ap()[b : b + 1, :].to_broadcast([128, T]))
        xna = work.tile([128, T], F16, tag="xna")
        nc.vector.tensor_mul(xna, xa[b], bc)
        xnb = work.tile([72, T], F16, tag="xnb")
        nc.vector.tensor_mul(xnb, xb[b], bc[0:72, :])

        pes = {}
        # s-major permuted stream: PSUM col j = s*48 + w  <->  t = 25*w + s,
        # so each "group" s is 48 contiguous windows-values; pooling becomes a
        # pairwise max/min tree over contiguous f16 slabs (DVE 2x mode).
        for pi, (p0, psz) in enumerate(P_TILES):
            ps_s = sim_ps.tile([psz, T], F32, tag="sim")
            for ni, (n0, nsz) in enumerate(N_TILES):
                xn = (xna if ni == 0 else xnb)
                for c, (c0, csz) in enumerate(T_CHUNKS):
                    nc.tensor.matmul(
                        ps_s[:, c0 : c0 + csz],
                        lhsT=patT16[ni][:, p0 : p0 + psz],
                        rhs=xn[:, c0 : c0 + csz],
                        start=(ni == 0), stop=(ni == 1),
                        skip_group_check=True,
                    )
            # drain with an s-major permuted OUT layout: sim16[p, s*48+w] =
            # psum[p, 25*w+s]; groups of s are then contiguous 48-wide slabs.
            sim16 = simp.tile([psz, T], F16, tag="sim16")
            nc.scalar.activation(
                out=sim16.rearrange("p (s w) -> p w s", w=W),
                in_=ps_s.rearrange("p (w s) -> p w s", s=STRIDE),
                func=AF.Copy)

            tr = simp.tile([psz, 1248], F16, tag="tr")
            sums = {}
            for pol, op in (("p", OP.max), ("m", OP.min)):
                nc.vector.tensor_tensor(tr[:, 0:576], sim16[:, 0:576],
                                        sim16[:, 576:1152], op=op)
                nc.vector.tensor_tensor(tr[:, 576:864], tr[:, 0:288],
                                        tr[:, 288:576], op=op)
                nc.vector.tensor_tensor(tr[:, 864:1008], tr[:, 576:720],
                                        tr[:, 720:864], op=op)
                nc.vector.tensor_tensor(tr[:, 1008:1104], tr[:, 864:960],
                                        tr[:, 912:1008], op=op)
                nc.vector.tensor_tensor(tr[:, 1104:1152], tr[:, 1008:1056],
                                        tr[:, 1056:1104], op=op)
                s32 = simp.tile([psz, 1], F32, tag=f"sum{pol}", name=f"s32{pol}{pi}")
                nc.vector.scalar_tensor_tensor(
                    out=tr[:, 1152:1200], in0=tr[:, 1104:1152], scalar=1.0,
                    in1=sim16[:, 1152:1200], op0=OP.bypass, op1=op,
                    accum_out=s32)
                sums[pol] = s32
            d = simp.tile([psz, 1], F32, tag="pesd")
            nc.vector.tensor_sub(d, sums["p"], sums["m"])
            pe = simp.tile([psz, 1], F32, tag="pes")
            nc.vector.tensor_scalar_mul(out=pe, in0=d, scalar1=isap[0:psz, b : b + 1])
            pes[pi] = pe

        skp = {}
        for pi, (p0, psz) in enumerate(P_TILES):
            sk = work.tile([psz, N], F16, tag=f"skp{pi}")
            nc.vector.tensor_scalar(out=sk, in0=top[pi], scalar1=pes[pi][:, 0:1],
                                    scalar2=None, op0=OP.mult)
            skp[pi] = sk

        for mi, (n0, nsz) in enumerate(N_TILES):
            ps_g = g_ps.tile([nsz, N], F32, tag="g")
            for pi, (p0, psz) in enumerate(P_TILES):
                nc.tensor.matmul(
                    ps_g,
                    lhsT=skp[pi][:, n0 : n0 + nsz],
                    rhs=kp16[pi][:, :],
                    start=(pi == 0), stop=(pi == len(P_TILES) - 1),
                )
            gm = work.tile([nsz, N], F32, tag=f"gout{mi}")
            nc.vector.tensor_mul(gm, ps_g, dmask[mi])
            nc.sync.dma_start(out=g_t.ap()[b, n0 : n0 + nsz, :], in_=gm)

    for g in range(BL // 4):
        for b in range(4 * g, 4 * g + 4):
            phase1_load(b)
        phase2_group(g)
        for b in range(4 * g, 4 * g + 4):
            phase3_sample(b)


_CACHE = {}
_LAST_RESULT = None


def _build():
    if "nc" in _CACHE:
        return _CACHE["nc"]
    from contextlib import ExitStack

    nc = bacc.Bacc("TRN2", target_bir_lowering=False, debug=False,
                   num_devices=NCORES)
    with ExitStack() as ctx:
        tc = ctx.enter_context(tile.TileContext(nc))
        _emit(nc, tc, ctx)
    nc.compile()
    _CACHE["nc"] = nc
    return nc


def kernel(x, patterns, length):
    """Full-input entry: x (64,200,1200) f32, patterns (400,200) f32,
    length (64,) int32 -> (graphs (64,200,200) f32, norm f32, orth f32)."""
    nc = _build()

    x16 = x.astype(np.float16).reshape(NCORES, BL, N, T)
    sap = np.floor(length.astype(np.float32) / STRIDE)
    isap2 = (1.0 / (2.0 * sap)).astype(np.float32).reshape(NCORES, BL)
    pat = np.ascontiguousarray(patterns.astype(np.float32))

    in_maps = []
    for c in range(NCORES):
        in_maps.append({
            "x16": np.ascontiguousarray(x16[c]),
            "patterns": pat,
            "invsap2": np.ascontiguousarray(
                np.broadcast_to(isap2[c][None, :], (128, BL))),
        })

    res = run_bass_kernel_spmd(nc, in_maps, core_ids=list(range(NCORES)))
    global _LAST_RESULT
    _LAST_RESULT = res
    graphs = np.concatenate([r["graphs"] for r in res.results], axis=0)
    norm = np.float32(res.results[0]["norm"].reshape(()))
    orth = np.float32(res.results[0]["orth"].reshape(()))
    return graphs, norm, orth


# revision 12
# speedup vs baseline: 1.4749x; 1.4749x over previous
"""Trainium2 Bass kernel for nn_KPFBNC (pattern-similarity graph + losses).

Computes, per batch sample b:
  graphs[b] = kp^T diag(PES[b]) kp * non_diag_scale
plus scalar losses (norm, orth) from the patterns alone.

Sharding: pure data parallelism over batch B=64 across 8 NeuronCores
(8 samples per core); patterns replicated; losses computed redundantly
on every core (taken from core 0).

Device pipeline per core:
  prep:   normalize patterns, top-3 mask, losses, PE-transpose -> patT
  per sample:
    xsq = x*x (DVE) -> column sums via ones-matmul (PE) ->
    invf = exp(-0.5*ln(fnorm2+eps)) (ACT, 4-sample batched rows) ->
    DMA broadcast of invf -> x_norm = x*invf (DVE) ->
    sim = patT^T @ x_norm (PE, fp16 in / f32 accum) ->
    ACT drains PSUM->SBUF fp16 -> windowed max/min reduces (DVE) ->
    PES -> graphs matmuls (PE) -> *non_diag_scale (DVE) -> DMA out
"""

import numpy as np
import ml_dtypes  # noqa: F401  (np float16 used; bf16 avail if needed)

import concourse.bass as bass
import concourse.bacc as bacc
import concourse.tile as tile
from concourse import mybir
from concourse.bass_utils import run_bass_kernel_spmd

# ---- problem constants (hardcoded per contract) ----
B, N, P, T = 64, 200, 400, 1200
TOPK, STRIDE = 3, 25
THRESHOLD = 0.3
EPS = 1e-9
NCORES = 8
BL = B // NCORES            # samples per core
W = T // STRIDE             # 48 windows
C_SCALE = TOPK * P / N      # 6.0 non-diag scale

F32 = mybir.dt.float32
F16 = mybir.dt.float16
AX = mybir.AxisListType
OP = mybir.AluOpType
AF = mybir.ActivationFunctionType

P_TILES = [(0, 128), (128, 128), (256, 128), (384, 16)]
N_TILES = [(0, 128), (128, 72)]
T_CHUNKS = [(0, 512), (512, 512), (1024, 176)]

BIG = 1e30


def _emit(nc, tc, ctx):
    # ---------------- IO ----------------
    x_t = nc.dram_tensor("x16", [BL, N, T], F16, kind="ExternalInput")
    pat_t = nc.dram_tensor("patterns", [P, N], F32, kind="ExternalInput")
    isap_t = nc.dram_tensor("invsap2", [128, BL], F32, kind="ExternalInput")
    g_t = nc.dram_tensor("graphs", [BL, N, N], F32, kind="ExternalOutput")
    norm_t = nc.dram_tensor("norm", [1, 1], F32, kind="ExternalOutput")
    orth_t = nc.dram_tensor("orth", [1, 1], F32, kind="ExternalOutput")
    invf_scr = nc.dram_tensor("invf_scr", [BL, T], F16, kind="Internal")

    # ---------------- inline consts ----------------
    ident_c = nc.inline_tensor(np.eye(128, dtype=np.float32), "identc")
    ones32_c = nc.inline_tensor(np.ones((128, 1), np.float32), "ones32c")
    ones16_c = nc.inline_tensor(np.ones((128, 1), np.float16), "ones16c")
    dmask_np = np.full((N, N), C_SCALE, np.float32) - np.eye(N, dtype=np.float32) * (
        C_SCALE - 1.0
    )
    dmask_c = nc.inline_tensor(dmask_np, "dmaskc")
    eyec_np = np.zeros((128, 16), np.float16)
    for _b in range(4):
        eyec_np[:, 4 * _b + _b] = 1.0
    eyec_c = nc.inline_tensor(eyec_np, "eyecolsc")

    statics = ctx.enter_context(tc.tile_pool(name="statics", bufs=1))

    ident = statics.tile([128, 128], F32, tag="ident")
    nc.sync.dma_start(out=ident, in_=ident_c.ap())
    ones32 = statics.tile([128, 1], F32, tag="ones32")
    nc.sync.dma_start(out=ones32, in_=ones32_c.ap())
    ones16 = statics.tile([128, 1], F16, tag="ones16")
    nc.sync.dma_start(out=ones16, in_=ones16_c.ap())
    eyecols = statics.tile([128, 16], F16, tag="eyecols")
    nc.sync.dma_start(out=eyecols, in_=eyec_c.ap())
    dmask = []
    for mi, (n0, nsz) in enumerate(N_TILES):
        dm = statics.tile([nsz, N], F32, tag=f"dmask{mi}")
        nc.sync.dma_start(out=dm, in_=dmask_c.ap()[n0 : n0 + nsz, :])
        dmask.append(dm)
    isap = statics.tile([128, BL], F32, tag="isap")
    nc.sync.dma_start(out=isap, in_=isap_t.ap())
    epsc = statics.tile([128, 1], F32, tag="epsc")
    nc.vector.memset(epsc, EPS)

    # =========================================================
    # PREP: patterns -> npat, top-3, losses, patT (f16 + f32)
    # =========================================================
    prep_cm = tc.tile_pool(name="prep", bufs=1)
    prep_ps_cm = tc.tile_pool(name="prep_ps", bufs=1, space="PSUM")
    loss_ps_cm = tc.tile_pool(name="loss_ps", bufs=1, space="PSUM")
    prep = prep_cm.__enter__()
    prep_ps = prep_ps_cm.__enter__()
    loss_ps = loss_ps_cm.__enter__()

    npat = []      # normalized patterns f32, per P tile
    top = []       # toppattern f32
    kp16 = []      # toppattern f16
    lenp2 = []     # per-row sum npat^2

    ps_norm = loss_ps.tile([1, 1], F32, tag="ps_norm")
    ps_orth = loss_ps.tile([1, 1], F32, tag="ps_orth")

    for pi, (p0, psz) in enumerate(P_TILES):
        pat = prep.tile([psz, N], F32, tag=f"pat{pi}")
        nc.sync.dma_start(out=pat, in_=pat_t.ap()[p0 : p0 + psz, :])

        scr = prep.tile([psz, N], F32, tag=f"scr{pi}")
        pn2 = prep.tile([psz, 1], F32, tag=f"pn2{pi}")
        nc.vector.scalar_tensor_tensor(
            out=scr, in0=pat, scalar=1.0, in1=pat, op0=OP.bypass, op1=OP.mult,
            accum_out=pn2,
        )
        slen = prep.tile([psz, 1], F32, tag=f"slen{pi}")
        nc.scalar.activation(out=slen, in_=pn2, func=AF.Sqrt,
                             bias=epsc[0:psz, 0:1], scale=1.0)
        invl = prep.tile([psz, 1], F32, tag=f"invl{pi}")
        nc.vector.reciprocal(out=invl, in_=slen)
        np_i = prep.tile([psz, N], F32, tag=f"npat{pi}")
        nc.vector.tensor_scalar_mul(out=np_i, in0=pat, scalar1=invl[:, 0:1])
        npat.append(np_i)

        # ---- top-3 by |npat| ----
        a = prep.tile([psz, N], F32, tag=f"abs{pi}")
        nc.scalar.activation(out=a, in_=np_i, func=AF.Abs)
        m1 = prep.tile([psz, 1], F32, tag=f"m1{pi}")
        nc.vector.reduce_max(out=m1, in_=a, axis=AX.X)
        e = prep.tile([psz, N], F32, tag=f"e{pi}")
        nc.vector.tensor_scalar(out=e, in0=a, scalar1=m1[:, 0:1], scalar2=None,
                                op0=OP.is_equal)
        a1 = prep.tile([psz, N], F32, tag=f"a1{pi}")
        nc.vector.scalar_tensor_tensor(
            out=a1, in0=e, scalar=-BIG, in1=a, op0=OP.mult, op1=OP.add)
        m2 = prep.tile([psz, 1], F32, tag=f"m2{pi}")
        nc.vector.reduce_max(out=m2, in_=a1, axis=AX.X)
        nc.vector.tensor_scalar(out=e, in0=a1, scalar1=m2[:, 0:1], scalar2=None,
                                op0=OP.is_equal)
        a2 = prep.tile([psz, N], F32, tag=f"a2{pi}")
        nc.vector.scalar_tensor_tensor(
            out=a2, in0=e, scalar=-BIG, in1=a1, op0=OP.mult, op1=OP.add)
        m3 = prep.tile([psz, 1], F32, tag=f"m3{pi}")
        nc.vector.reduce_max(out=m3, in_=a2, axis=AX.X)
        mask = prep.tile([psz, N], F32, tag=f"mask{pi}")
        nc.vector.tensor_scalar(out=mask, in0=a, scalar1=m3[:, 0:1], scalar2=None,
                                op0=OP.is_ge)
        t_i = statics.tile([psz, N], F32, tag=f"top{pi}")
        nc.vector.tensor_mul(t_i, mask, np_i)
        top.append(t_i)
        k_i = statics.tile([psz, N], F16, tag=f"kp16{pi}")
        nc.vector.tensor_copy(out=k_i, in_=t_i)
        kp16.append(k_i)

        # ---- norm loss pieces ----
        l2 = prep.tile([psz, 1], F32, tag=f"lenp2{pi}")
        nc.vector.scalar_tensor_tensor(
            out=scr, in0=np_i, scalar=1.0, in1=np_i, op0=OP.bypass, op1=OP.mult,
            accum_out=l2)
        lenp2.append(l2)
        q = prep.tile([psz, 1], F32, tag=f"q{pi}")
        nc.vector.scalar_tensor_tensor(
            out=scr, in0=t_i, scalar=1.0, in1=t_i, op0=OP.bypass, op1=OP.mult,
            accum_out=q)
        il2 = prep.tile([psz, 1], F32, tag=f"il2{pi}")
        nc.vector.reciprocal(out=il2, in_=l2)
        r2 = prep.tile([psz, 1], F32, tag=f"r2{pi}")
        nc.vector.tensor_mul(r2, q, il2)
        r = prep.tile([psz, 1], F32, tag=f"r{pi}")
        nc.scalar.activation(out=r, in_=r2, func=AF.Sqrt)
        tm = prep.tile([psz, 1], F32, tag=f"tm{pi}")
        nc.vector.tensor_scalar(out=tm, in0=r, scalar1=-1.0, scalar2=1.0,
                                op0=OP.mult, op1=OP.add)
        tm2 = prep.tile([psz, 1], F32, tag=f"tm2{pi}")
        nc.vector.tensor_mul(tm2, tm, tm)
        nc.tensor.matmul(ps_norm[0:1, 0:1], lhsT=tm2[:, 0:1], rhs=ones32[0:psz, 0:1],
                         start=(pi == 0), stop=(pi == len(P_TILES) - 1),
                         skip_group_check=True)

    # ---- transpose npat -> patT (two N-row tiles of (nsz, P)) ----
    patT16 = []
    patT32 = []
    for ni, (n0, nsz) in enumerate(N_TILES):
        ps_pt = prep_ps.tile([nsz, P], F32, tag=f"ps_patT{ni}")
        for pi, (p0, psz) in enumerate(P_TILES):
            nc.tensor.transpose(
                ps_pt[0:nsz, p0 : p0 + psz],
                npat[pi][:, n0 : n0 + nsz],
                ident[0:psz, 0:psz],
            )
        pt32 = statics.tile([nsz, P], F32, tag=f"patT32_{ni}")
        nc.scalar.copy(out=pt32, in_=ps_pt)
        pt16 = statics.tile([nsz, P], F16, tag=f"patT16_{ni}")
        nc.vector.tensor_copy(out=pt16, in_=ps_pt)
        patT16.append(pt16)
        patT32.append(pt32)

    # ---- orth loss ----
    for pi, (p0, psz) in enumerate(P_TILES):
        ps_o = prep_ps.tile([psz, P], F32, tag="ps_simpp")
        for ni, (n0, nsz) in enumerate(N_TILES):
            nc.tensor.matmul(ps_o, lhsT=patT32[ni][:, p0 : p0 + psz],
                             rhs=patT32[ni][:, :],
                             start=(ni == 0), stop=(ni == 1))
        ab = prep.tile([psz, P], F32, tag=f"oab{pi}")
        nc.scalar.activation(out=ab, in_=ps_o, func=AF.Abs)
        z = prep.tile([psz, P], F32, tag=f"oz{pi}")
        nc.vector.tensor_scalar(out=z, in0=ab, scalar1=-THRESHOLD,
                                scalar2=1.0 / (1.0 - THRESHOLD + 1e-5),
                                op0=OP.add, op1=OP.mult)
        rl = prep.tile([psz, P], F32, tag=f"orl{pi}")
        nc.vector.tensor_scalar_max(out=rl, in0=z, scalar1=0.0)
        oscr = prep.tile([psz, P], F32, tag=f"oscr{pi}")
        osum = prep.tile([psz, 1], F32, tag=f"osum{pi}")
        nc.vector.scalar_tensor_tensor(
            out=oscr, in0=rl, scalar=1.0, in1=rl, op0=OP.bypass, op1=OP.mult,
            accum_out=osum)
        # subtract the diagonal contribution relu((lenp2 - th)/(1-th+1e-5))^2
        zd = prep.tile([psz, 1], F32, tag=f"ozd{pi}")
        nc.vector.tensor_scalar(out=zd, in0=lenp2[pi], scalar1=-THRESHOLD,
                                scalar2=1.0 / (1.0 - THRESHOLD + 1e-5),
                                op0=OP.add, op1=OP.mult)
        nc.vector.tensor_scalar_max(out=zd, in0=zd, scalar1=0.0)
        d2 = prep.tile([psz, 1], F32, tag=f"od2{pi}")
        nc.vector.tensor_mul(d2, zd, zd)
        nc.vector.tensor_sub(osum, osum, d2)
        nc.tensor.matmul(ps_orth[0:1, 0:1], lhsT=osum[:, 0:1],
                         rhs=ones32[0:psz, 0:1],
                         start=(pi == 0), stop=(pi == len(P_TILES) - 1),
                         skip_group_check=True)

    normv = prep.tile([1, 1], F32, tag="normv")
    nc.scalar.mul(out=normv, in_=ps_norm, mul=1.0 / P)
    nc.sync.dma_start(out=norm_t.ap(), in_=normv)
    orthv = prep.tile([1, 1], F32, tag="orthv")
    nc.scalar.mul(out=orthv, in_=ps_orth, mul=1.0 / (P * P))
    nc.sync.dma_start(out=orth_t.ap(), in_=orthv)

    loss_ps_cm.__exit__(None, None, None)
    prep_ps_cm.__exit__(None, None, None)
    prep_cm.__exit__(None, None, None)

    # =========================================================
    # PER-SAMPLE PIPELINE
    # =========================================================
    xp = ctx.enter_context(tc.tile_pool(name="xp", bufs=5))
    sq = ctx.enter_context(tc.tile_pool(name="sq", bufs=5))
    rows = ctx.enter_context(tc.tile_pool(name="rows", bufs=2))
    work = ctx.enter_context(tc.tile_pool(name="work", bufs=2))
    simp = ctx.enter_context(tc.tile_pool(name="simp", bufs=3))
    fn_ps = ctx.enter_context(tc.tile_pool(name="fn_ps", bufs=1, space="PSUM"))
    sim_ps = ctx.enter_context(tc.tile_pool(name="sim_ps", bufs=5, space="PSUM"))
    g_ps = ctx.enter_context(tc.tile_pool(name="g_ps", bufs=1, space="PSUM"))

    xa = {}
    xb = {}
    xsqa = {}
    xsqb = {}

    def phase1_load(b):
        xa[b] = xp.tile([128, T], F16, tag="xa", name=f"xa{b}")
        nc.sync.dma_start(out=xa[b], in_=x_t.ap()[b, 0:128, :])
        xb[b] = xp.tile([72, T], F16, tag="xb", name=f"xb{b}")
        nc.sync.dma_start(out=xb[b], in_=x_t.ap()[b, 128:200, :])
        xsqa[b] = sq.tile([128, T], F16, tag="xsqa", name=f"xsqa{b}")
        nc.gpsimd.tensor_mul(xsqa[b], xa[b], xa[b])
        xsqb[b] = sq.tile([72, T], F16, tag="xsqb", name=f"xsqb{b}")
        nc.gpsimd.tensor_mul(xsqb[b], xb[b], xb[b])

    def phase2_group(g):
        grp = range(4 * g, 4 * g + 4)
        lnf = rows.tile([4, T], F32, tag="lnf")
        for c, (c0, csz) in enumerate(T_CHUNKS):
            fn = fn_ps.tile([4, 512], F32, tag="fn")
            for k, b in enumerate(grp):
                for ni, (n0, nsz) in enumerate(N_TILES):
                    xs = (xsqa[b] if ni == 0 else xsqb[b])
                    nc.tensor.matmul(
                        fn[0:4, 0:csz],
                        lhsT=eyecols[0:nsz, 4 * k : 4 * k + 4],
                        rhs=xs[:, c0 : c0 + csz],
                        start=(k == 0 and ni == 0), stop=(k == 3 and ni == 1),
                        skip_group_check=True,
                    )
            nc.scalar.activation(
                out=lnf[0:4, c0 : c0 + csz],
                in_=fn[0:4, 0:csz],
                func=AF.Ln, bias=epsc[0:4, 0:1], scale=1.0,
            )
        invf = rows.tile([4, T], F16, tag="invf")
        nc.scalar.activation(out=invf[0:4, :], in_=lnf[0:4, :],
                             func=AF.Exp, scale=-0.5)
        for k, b in enumerate(grp):
            nc.sync.dma_start(out=invf_scr.ap()[b : b + 1, :],
                              in_=invf[k : k + 1, :])

    def phase3_sample(b):
        bc = work.tile([128, T], F16, tag="bc")
        nc.sync.dma_start(out=bc, in_=invf_scr.ap()[b : b + 1, :].to_broadcast([128, T]))
        xna = work.tile([128, T], F16, tag="xna")
        nc.vector.tensor_mul(xna, xa[b], bc)
        xnb = work.tile([72, T], F16, tag="xnb")
        nc.vector.tensor_mul(xnb, xb[b], bc[0:72, :])

        pes = {}
        # s-major permuted stream: PSUM col j = s*48 + w  <->  t = 25*w + s,
        # so each "group" s is 48 contiguous windows-values; pooling becomes a
        # pairwise max/min tree over contiguous f16 slabs (DVE 2x mode).
        for pi, (p0, psz) in enumerate(P_TILES):
            sim16 = simp.tile([psz, T], F16, tag="sim16")
            for c, (c0, csz) in enumerate(T_CHUNKS):
                ps_c = sim_ps.tile([psz, 512], F32, tag="sim", name=f"psc{pi}{c}")
                for ni, (n0, nsz) in enumerate(N_TILES):
                    xn = (xna if ni == 0 else xnb)
                    nc.tensor.matmul(
                        ps_c[:, 0:csz],
                        lhsT=patT16[ni][:, p0 : p0 + psz],
                        rhs=xn[:, c0 : c0 + csz],
                        start=(ni == 0), stop=(ni == 1),
                        skip_group_check=True,
                    )
                nc.scalar.activation(out=sim16[:, c0 : c0 + csz],
                                     in_=ps_c[:, 0:csz], func=AF.Copy)
            simw = sim16.rearrange("p (w s) -> p w s", s=STRIDE)
            pos = simp.tile([psz, W], F16, tag="pos")
            nc.vector.reduce_max(out=pos, in_=simw, axis=AX.X)
            neg = simp.tile([psz, W], F16, tag="neg")
            nc.vector.tensor_reduce(out=neg, in_=simw, axis=AX.X, op=OP.min)
            scr48 = simp.tile([psz, W], F16, tag="scr48")
            d = simp.tile([psz, 1], F32, tag="pesd")
            nc.vector.scalar_tensor_tensor(
                out=scr48, in0=pos, scalar=1.0, in1=neg,
                op0=OP.bypass, op1=OP.subtract, accum_out=d)
            pe = simp.tile([psz, 1], F32, tag="pes")
            nc.vector.tensor_scalar_mul(out=pe, in0=d, scalar1=isap[0:psz, b : b + 1])
            pes[pi] = pe

        skp = {}
        for pi, (p0, psz) in enumerate(P_TILES):
            sk = work.tile([psz, N], F16, tag=f"skp{pi}")
            nc.scalar.activation(out=sk, in_=top[pi], func=AF.Copy,
                                 scale=pes[pi][:, 0:1])
            skp[pi] = sk

        for mi, (n0, nsz) in enumerate(N_TILES):
            ps_g = g_ps.tile([nsz, N], F32, tag="g")
            for pi, (p0, psz) in enumerate(P_TILES):
                nc.tensor.matmul(
                    ps_g,
                    lhsT=skp[pi][:, n0 : n0 + nsz],
                    rhs=kp16[pi][:, :],
                    start=(pi == 0), stop=(pi == len(P_TILES) - 1),
                )
            gm = work.tile([nsz, N], F32, tag=f"gout{mi}")
            nc.vector.tensor_mul(gm, ps_g, dmask[mi])
            nc.sync.dma_start(out=g_t.ap()[b, n0 : n0 + nsz, :], in_=gm)

    for g in range(BL // 4):
        for b in range(4 * g, 4 * g + 4):
            phase1_load(b)
        phase2_group(g)
        for b in range(4 * g, 4 * g + 4):
            phase3_sample(b)


_CACHE = {}
_LAST_RESULT = None


def _build():
    if "nc" in _CACHE:
        return _CACHE["nc"]
    from contextlib import ExitStack

    nc = bacc.Bacc("TRN2", target_bir_lowering=False, debug=False,
                   num_devices=NCORES)
    with ExitStack() as ctx:
        tc = ctx.enter_context(tile.TileContext(nc))
        _emit(nc, tc, ctx)
    nc.compile()
    _CACHE["nc"] = nc
    return nc


def kernel(x, patterns, length):
    """Full-input entry: x (64,200,1200) f32, patterns (400,200) f32,
    length (64,) int32 -> (graphs (64,200,200) f32, norm f32, orth f32)."""
    nc = _build()

    x16 = x.astype(np.float16).reshape(NCORES, BL, N, T)
    sap = np.floor(length.astype(np.float32) / STRIDE)
    isap2 = (1.0 / (2.0 * sap)).astype(np.float32).reshape(NCORES, BL)
    pat = np.ascontiguousarray(patterns.astype(np.float32))

    in_maps = []
    for c in range(NCORES):
        in_maps.append({
            "x16": np.ascontiguousarray(x16[c]),
            "patterns": pat,
            "invsap2": np.ascontiguousarray(
                np.broadcast_to(isap2[c][None, :], (128, BL))),
        })

    res = run_bass_kernel_spmd(nc, in_maps, core_ids=list(range(NCORES)))
    global _LAST_RESULT
    _LAST_RESULT = res
    graphs = np.concatenate([r["graphs"] for r in res.results], axis=0)
    norm = np.float32(res.results[0]["norm"].reshape(()))
    orth = np.float32(res.results[0]["orth"].reshape(()))
    return graphs, norm, orth


# revision 13
# speedup vs baseline: 1.5031x; 1.0191x over previous
"""Trainium2 Bass kernel for nn_KPFBNC (pattern-similarity graph + losses).

Computes, per batch sample b:
  graphs[b] = kp^T diag(PES[b]) kp * non_diag_scale
plus scalar losses (norm, orth) from the patterns alone.

Sharding: pure data parallelism over batch B=64 across 8 NeuronCores
(8 samples per core); patterns replicated; losses computed redundantly
on every core (taken from core 0).

Device pipeline per core:
  prep:   normalize patterns, top-3 mask, losses, PE-transpose -> patT
  per sample:
    xsq = x*x (DVE) -> column sums via ones-matmul (PE) ->
    invf = exp(-0.5*ln(fnorm2+eps)) (ACT, 4-sample batched rows) ->
    DMA broadcast of invf -> x_norm = x*invf (DVE) ->
    sim = patT^T @ x_norm (PE, fp16 in / f32 accum) ->
    ACT drains PSUM->SBUF fp16 -> windowed max/min reduces (DVE) ->
    PES -> graphs matmuls (PE) -> *non_diag_scale (DVE) -> DMA out
"""

import numpy as np
import ml_dtypes  # noqa: F401  (np float16 used; bf16 avail if needed)

import concourse.bass as bass
import concourse.bacc as bacc
import concourse.tile as tile
from concourse import mybir
from concourse.bass_utils import run_bass_kernel_spmd

# ---- problem constants (hardcoded per contract) ----
B, N, P, T = 64, 200, 400, 1200
TOPK, STRIDE = 3, 25
THRESHOLD = 0.3
EPS = 1e-9
NCORES = 8
BL = B // NCORES            # samples per core
W = T // STRIDE             # 48 windows
C_SCALE = TOPK * P / N      # 6.0 non-diag scale

F32 = mybir.dt.float32
F16 = mybir.dt.float16
AX = mybir.AxisListType
OP = mybir.AluOpType
AF = mybir.ActivationFunctionType

P_TILES = [(0, 128), (128, 128), (256, 128), (384, 16)]
N_TILES = [(0, 128), (128, 72)]
T_CHUNKS = [(0, 512), (512, 512), (1024, 176)]

BIG = 1e30


def _emit(nc, tc, ctx):
    # ---------------- IO ----------------
    x_t = nc.dram_tensor("x16", [BL, N, T], F16, kind="ExternalInput")
    pat_t = nc.dram_tensor("patterns", [P, N], F32, kind="ExternalInput")
    isap_t = nc.dram_tensor("invsap2", [128, BL], F32, kind="ExternalInput")
    g_t = nc.dram_tensor("graphs", [BL, N, N], F32, kind="ExternalOutput")
    norm_t = nc.dram_tensor("norm", [1, 1], F32, kind="ExternalOutput")
    orth_t = nc.dram_tensor("orth", [1, 1], F32, kind="ExternalOutput")
    invf_scr = nc.dram_tensor("invf_scr", [BL, T], F16, kind="Internal")

    # ---------------- inline consts ----------------
    ident_c = nc.inline_tensor(np.eye(128, dtype=np.float32), "identc")
    ones32_c = nc.inline_tensor(np.ones((128, 1), np.float32), "ones32c")
    ones16_c = nc.inline_tensor(np.ones((128, 1), np.float16), "ones16c")
    dmask_np = np.full((N, N), C_SCALE, np.float32) - np.eye(N, dtype=np.float32) * (
        C_SCALE - 1.0
    )
    dmask_c = nc.inline_tensor(dmask_np, "dmaskc")
    eyec_np = np.zeros((128, 16), np.float16)
    for _b in range(4):
        eyec_np[:, 4 * _b + _b] = 1.0
    eyec_c = nc.inline_tensor(eyec_np, "eyecolsc")

    statics = ctx.enter_context(tc.tile_pool(name="statics", bufs=1))

    ident = statics.tile([128, 128], F32, tag="ident")
    nc.sync.dma_start(out=ident, in_=ident_c.ap())
    ones32 = statics.tile([128, 1], F32, tag="ones32")
    nc.sync.dma_start(out=ones32, in_=ones32_c.ap())
    ones16 = statics.tile([128, 1], F16, tag="ones16")
    nc.sync.dma_start(out=ones16, in_=ones16_c.ap())
    eyecols = statics.tile([128, 16], F16, tag="eyecols")
    nc.sync.dma_start(out=eyecols, in_=eyec_c.ap())
    dmask = []
    for mi, (n0, nsz) in enumerate(N_TILES):
        dm = statics.tile([nsz, N], F32, tag=f"dmask{mi}")
        nc.sync.dma_start(out=dm, in_=dmask_c.ap()[n0 : n0 + nsz, :])
        dmask.append(dm)
    isap = statics.tile([128, BL], F32, tag="isap")
    nc.sync.dma_start(out=isap, in_=isap_t.ap())
    epsc = statics.tile([128, 1], F32, tag="epsc")
    nc.vector.memset(epsc, EPS)

    # =========================================================
    # PREP: patterns -> npat, top-3, losses, patT (f16 + f32)
    # =========================================================
    prep_cm = tc.tile_pool(name="prep", bufs=1)
    prep_ps_cm = tc.tile_pool(name="prep_ps", bufs=1, space="PSUM")
    loss_ps_cm = tc.tile_pool(name="loss_ps", bufs=1, space="PSUM")
    prep = prep_cm.__enter__()
    prep_ps = prep_ps_cm.__enter__()
    loss_ps = loss_ps_cm.__enter__()

    npat = []      # normalized patterns f32, per P tile
    top = []       # toppattern f32
    kp16 = []      # toppattern f16
    lenp2 = []     # per-row sum npat^2

    ps_norm = loss_ps.tile([1, 1], F32, tag="ps_norm")
    ps_orth = loss_ps.tile([1, 1], F32, tag="ps_orth")

    for pi, (p0, psz) in enumerate(P_TILES):
        pat = prep.tile([psz, N], F32, tag=f"pat{pi}")
        nc.sync.dma_start(out=pat, in_=pat_t.ap()[p0 : p0 + psz, :])

        scr = prep.tile([psz, N], F32, tag=f"scr{pi}")
        pn2 = prep.tile([psz, 1], F32, tag=f"pn2{pi}")
        nc.vector.scalar_tensor_tensor(
            out=scr, in0=pat, scalar=1.0, in1=pat, op0=OP.bypass, op1=OP.mult,
            accum_out=pn2,
        )
        slen = prep.tile([psz, 1], F32, tag=f"slen{pi}")
        nc.scalar.activation(out=slen, in_=pn2, func=AF.Sqrt,
                             bias=epsc[0:psz, 0:1], scale=1.0)
        invl = prep.tile([psz, 1], F32, tag=f"invl{pi}")
        nc.vector.reciprocal(out=invl, in_=slen)
        np_i = prep.tile([psz, N], F32, tag=f"npat{pi}")
        nc.vector.tensor_scalar_mul(out=np_i, in0=pat, scalar1=invl[:, 0:1])
        npat.append(np_i)

        # ---- top-3 by |npat| ----
        a = prep.tile([psz, N], F32, tag=f"abs{pi}")
        nc.scalar.activation(out=a, in_=np_i, func=AF.Abs)
        m1 = prep.tile([psz, 1], F32, tag=f"m1{pi}")
        nc.vector.reduce_max(out=m1, in_=a, axis=AX.X)
        e = prep.tile([psz, N], F32, tag=f"e{pi}")
        nc.vector.tensor_scalar(out=e, in0=a, scalar1=m1[:, 0:1], scalar2=None,
                                op0=OP.is_equal)
        a1 = prep.tile([psz, N], F32, tag=f"a1{pi}")
        nc.vector.scalar_tensor_tensor(
            out=a1, in0=e, scalar=-BIG, in1=a, op0=OP.mult, op1=OP.add)
        m2 = prep.tile([psz, 1], F32, tag=f"m2{pi}")
        nc.vector.reduce_max(out=m2, in_=a1, axis=AX.X)
        nc.vector.tensor_scalar(out=e, in0=a1, scalar1=m2[:, 0:1], scalar2=None,
                                op0=OP.is_equal)
        a2 = prep.tile([psz, N], F32, tag=f"a2{pi}")
        nc.vector.scalar_tensor_tensor(
            out=a2, in0=e, scalar=-BIG, in1=a1, op0=OP.mult, op1=OP.add)
        m3 = prep.tile([psz, 1], F32, tag=f"m3{pi}")
        nc.vector.reduce_max(out=m3, in_=a2, axis=AX.X)
        mask = prep.tile([psz, N], F32, tag=f"mask{pi}")
        nc.vector.tensor_scalar(out=mask, in0=a, scalar1=m3[:, 0:1], scalar2=None,
                                op0=OP.is_ge)
        t_i = statics.tile([psz, N], F32, tag=f"top{pi}")
        nc.vector.tensor_mul(t_i, mask, np_i)
        top.append(t_i)
        k_i = statics.tile([psz, N], F16, tag=f"kp16{pi}")
        nc.vector.tensor_copy(out=k_i, in_=t_i)
        kp16.append(k_i)

        # ---- norm loss pieces ----
        l2 = prep.tile([psz, 1], F32, tag=f"lenp2{pi}")
        nc.vector.scalar_tensor_tensor(
            out=scr, in0=np_i, scalar=1.0, in1=np_i, op0=OP.bypass, op1=OP.mult,
            accum_out=l2)
        lenp2.append(l2)
        q = prep.tile([psz, 1], F32, tag=f"q{pi}")
        nc.vector.scalar_tensor_tensor(
            out=scr, in0=t_i, scalar=1.0, in1=t_i, op0=OP.bypass, op1=OP.mult,
            accum_out=q)
        il2 = prep.tile([psz, 1], F32, tag=f"il2{pi}")
        nc.vector.reciprocal(out=il2, in_=l2)
        r2 = prep.tile([psz, 1], F32, tag=f"r2{pi}")
        nc.vector.tensor_mul(r2, q, il2)
        r = prep.tile([psz, 1], F32, tag=f"r{pi}")
        nc.scalar.activation(out=r, in_=r2, func=AF.Sqrt)
        tm = prep.tile([psz, 1], F32, tag=f"tm{pi}")
        nc.vector.tensor_scalar(out=tm, in0=r, scalar1=-1.0, scalar2=1.0,
                                op0=OP.mult, op1=OP.add)
        tm2 = prep.tile([psz, 1], F32, tag=f"tm2{pi}")
        nc.vector.tensor_mul(tm2, tm, tm)
        nc.tensor.matmul(ps_norm[0:1, 0:1], lhsT=tm2[:, 0:1], rhs=ones32[0:psz, 0:1],
                         start=(pi == 0), stop=(pi == len(P_TILES) - 1),
                         skip_group_check=True)

    # ---- transpose npat -> patT (two N-row tiles of (nsz, P)) ----
    patT16 = []
    patT32 = []
    for ni, (n0, nsz) in enumerate(N_TILES):
        ps_pt = prep_ps.tile([nsz, P], F32, tag=f"ps_patT{ni}")
        for pi, (p0, psz) in enumerate(P_TILES):
            nc.tensor.transpose(
                ps_pt[0:nsz, p0 : p0 + psz],
                npat[pi][:, n0 : n0 + nsz],
                ident[0:psz, 0:psz],
            )
        pt32 = statics.tile([nsz, P], F32, tag=f"patT32_{ni}")
        nc.scalar.copy(out=pt32, in_=ps_pt)
        pt16 = statics.tile([nsz, P], F16, tag=f"patT16_{ni}")
        nc.vector.tensor_copy(out=pt16, in_=ps_pt)
        patT16.append(pt16)
        patT32.append(pt32)

    # ---- orth loss ----
    for pi, (p0, psz) in enumerate(P_TILES):
        ps_o = prep_ps.tile([psz, P], F32, tag="ps_simpp")
        for ni, (n0, nsz) in enumerate(N_TILES):
            nc.tensor.matmul(ps_o, lhsT=patT32[ni][:, p0 : p0 + psz],
                             rhs=patT32[ni][:, :],
                             start=(ni == 0), stop=(ni == 1))
        ab = prep.tile([psz, P], F32, tag=f"oab{pi}")
        nc.scalar.activation(out=ab, in_=ps_o, func=AF.Abs)
        z = prep.tile([psz, P], F32, tag=f"oz{pi}")
        nc.vector.tensor_scalar(out=z, in0=ab, scalar1=-THRESHOLD,
                                scalar2=1.0 / (1.0 - THRESHOLD + 1e-5),
                                op0=OP.add, op1=OP.mult)
        rl = prep.tile([psz, P], F32, tag=f"orl{pi}")
        nc.vector.tensor_scalar_max(out=rl, in0=z, scalar1=0.0)
        oscr = prep.tile([psz, P], F32, tag=f"oscr{pi}")
        osum = prep.tile([psz, 1], F32, tag=f"osum{pi}")
        nc.vector.scalar_tensor_tensor(
            out=oscr, in0=rl, scalar=1.0, in1=rl, op0=OP.bypass, op1=OP.mult,
            accum_out=osum)
        # subtract the diagonal contribution relu((lenp2 - th)/(1-th+1e-5))^2
        zd = prep.tile([psz, 1], F32, tag=f"ozd{pi}")
        nc.vector.tensor_scalar(out=zd, in0=lenp2[pi], scalar1=-THRESHOLD,
                                scalar2=1.0 / (1.0 - THRESHOLD + 1e-5),
                                op0=OP.add, op1=OP.mult)
        nc.vector.tensor_scalar_max(out=zd, in0=zd, scalar1=0.0)
        d2 = prep.tile([psz, 1], F32, tag=f"od2{pi}")
        nc.vector.tensor_mul(d2, zd, zd)
        nc.vector.tensor_sub(osum, osum, d2)
        nc.tensor.matmul(ps_orth[0:1, 0:1], lhsT=osum[:, 0:1],
                         rhs=ones32[0:psz, 0:1],
                         start=(pi == 0), stop=(pi == len(P_TILES) - 1),
                         skip_group_check=True)

    normv = prep.tile([1, 1], F32, tag="normv")
    nc.scalar.mul(out=normv, in_=ps_norm, mul=1.0 / P)
    nc.sync.dma_start(out=norm_t.ap(), in_=normv)
    orthv = prep.tile([1, 1], F32, tag="orthv")
    nc.scalar.mul(out=orthv, in_=ps_orth, mul=1.0 / (P * P))
    nc.sync.dma_start(out=orth_t.ap(), in_=orthv)

    loss_ps_cm.__exit__(None, None, None)
    prep_ps_cm.__exit__(None, None, None)
    prep_cm.__exit__(None, None, None)

    # =========================================================
    # PER-SAMPLE PIPELINE
    # =========================================================
    xp = ctx.enter_context(tc.tile_pool(name="xp", bufs=8))
    sq = ctx.enter_context(tc.tile_pool(name="sq", bufs=8))
    rows = ctx.enter_context(tc.tile_pool(name="rows", bufs=2))
    work = ctx.enter_context(tc.tile_pool(name="work", bufs=2))
    simp = ctx.enter_context(tc.tile_pool(name="simp", bufs=3))
    fn_ps = ctx.enter_context(tc.tile_pool(name="fn_ps", bufs=2, space="PSUM"))
    sim_ps = ctx.enter_context(tc.tile_pool(name="sim_ps", bufs=5, space="PSUM"))
    g_ps = ctx.enter_context(tc.tile_pool(name="g_ps", bufs=1, space="PSUM"))

    xa = {}
    xb = {}
    xsqa = {}
    xsqb = {}

    def phase1_load(b):
        xa[b] = xp.tile([128, T], F16, tag="xa", name=f"xa{b}")
        nc.sync.dma_start(out=xa[b], in_=x_t.ap()[b, 0:128, :])
        xb[b] = xp.tile([72, T], F16, tag="xb", name=f"xb{b}")
        nc.sync.dma_start(out=xb[b], in_=x_t.ap()[b, 128:200, :])
        xsqa[b] = sq.tile([128, T], F16, tag="xsqa", name=f"xsqa{b}")
        nc.gpsimd.tensor_mul(xsqa[b], xa[b], xa[b])
        xsqb[b] = sq.tile([72, T], F16, tag="xsqb", name=f"xsqb{b}")
        nc.gpsimd.tensor_mul(xsqb[b], xb[b], xb[b])

    def phase2_group(g):
        grp = range(4 * g, 4 * g + 4)
        lnf = rows.tile([4, T], F32, tag="lnf")
        for c, (c0, csz) in enumerate(T_CHUNKS):
            fn = fn_ps.tile([4, 512], F32, tag="fn")
            for k, b in enumerate(grp):
                for ni, (n0, nsz) in enumerate(N_TILES):
                    xs = (xsqa[b] if ni == 0 else xsqb[b])
                    nc.tensor.matmul(
                        fn[0:4, 0:csz],
                        lhsT=eyecols[0:nsz, 4 * k : 4 * k + 4],
                        rhs=xs[:, c0 : c0 + csz],
                        start=(k == 0 and ni == 0), stop=(k == 3 and ni == 1),
                        skip_group_check=True,
                    )
            nc.scalar.activation(
                out=lnf[0:4, c0 : c0 + csz],
                in_=fn[0:4, 0:csz],
                func=AF.Ln, bias=epsc[0:4, 0:1], scale=1.0,
            )
        invf = rows.tile([4, T], F16, tag="invf")
        nc.scalar.activation(out=invf[0:4, :], in_=lnf[0:4, :],
                             func=AF.Exp, scale=-0.5)
        for k, b in enumerate(grp):
            nc.sync.dma_start(out=invf_scr.ap()[b : b + 1, :],
                              in_=invf[k : k + 1, :])

    def phase3_sample(b):
        bc = work.tile([128, T], F16, tag="bc")
        nc.sync.dma_start(out=bc, in_=invf_scr.ap()[b : b + 1, :].to_broadcast([128, T]))
        xna = work.tile([128, T], F16, tag="xna")
        nc.vector.tensor_mul(xna, xa[b], bc)
        xnb = work.tile([72, T], F16, tag="xnb")
        nc.vector.tensor_mul(xnb, xb[b], bc[0:72, :])

        pes = {}
        # s-major permuted stream: PSUM col j = s*48 + w  <->  t = 25*w + s,
        # so each "group" s is 48 contiguous windows-values; pooling becomes a
        # pairwise max/min tree over contiguous f16 slabs (DVE 2x mode).
        for pi, (p0, psz) in enumerate(P_TILES):
            sim16 = simp.tile([psz, T], F16, tag="sim16")
            for c, (c0, csz) in enumerate(T_CHUNKS):
                ps_c = sim_ps.tile([psz, 512], F32, tag="sim", name=f"psc{pi}{c}")
                for ni, (n0, nsz) in enumerate(N_TILES):
                    xn = (xna if ni == 0 else xnb)
                    nc.tensor.matmul(
                        ps_c[:, 0:csz],
                        lhsT=patT16[ni][:, p0 : p0 + psz],
                        rhs=xn[:, c0 : c0 + csz],
                        start=(ni == 0), stop=(ni == 1),
                        skip_group_check=True,
                    )
                nc.scalar.activation(out=sim16[:, c0 : c0 + csz],
                                     in_=ps_c[:, 0:csz], func=AF.Copy)
            simw = sim16.rearrange("p (w s) -> p w s", s=STRIDE)
            pos = simp.tile([psz, W], F16, tag="pos")
            nc.vector.reduce_max(out=pos, in_=simw, axis=AX.X)
            neg = simp.tile([psz, W], F16, tag="neg")
            nc.vector.tensor_reduce(out=neg, in_=simw, axis=AX.X, op=OP.min)
            scr48 = simp.tile([psz, W], F16, tag="scr48")
            d = simp.tile([psz, 1], F32, tag="pesd")
            nc.vector.scalar_tensor_tensor(
                out=scr48, in0=pos, scalar=1.0, in1=neg,
                op0=OP.bypass, op1=OP.subtract, accum_out=d)
            pe = simp.tile([psz, 1], F32, tag="pes")
            nc.vector.tensor_scalar_mul(out=pe, in0=d, scalar1=isap[0:psz, b : b + 1])
            pes[pi] = pe

        skp = {}
        for pi, (p0, psz) in enumerate(P_TILES):
            sk = work.tile([psz, N], F16, tag=f"skp{pi}")
            nc.scalar.activation(out=sk, in_=top[pi], func=AF.Copy,
                                 scale=pes[pi][:, 0:1])
            skp[pi] = sk

        for mi, (n0, nsz) in enumerate(N_TILES):
            ps_g = g_ps.tile([nsz, N], F32, tag="g")
            for pi, (p0, psz) in enumerate(P_TILES):
                nc.tensor.matmul(
                    ps_g,
                    lhsT=skp[pi][:, n0 : n0 + nsz],
                    rhs=kp16[pi][:, :],
                    start=(pi == 0), stop=(pi == len(P_TILES) - 1),
                )
            gm = work.tile([nsz, N], F32, tag=f"gout{mi}")
            nc.vector.tensor_mul(gm, ps_g, dmask[mi])
            nc.sync.dma_start(out=g_t.ap()[b, n0 : n0 + nsz, :], in_=gm)

    for g in range(BL // 4):
        for b in range(4 * g, 4 * g + 4):
            phase1_load(b)
        phase2_group(g)
    for b in range(BL):
        phase3_sample(b)


_CACHE = {}
_LAST_RESULT = None


def _build():
    if "nc" in _CACHE:
        return _CACHE["nc"]
    from contextlib import ExitStack

    nc = bacc.Bacc("TRN2", target_bir_lowering=False, debug=False,
                   num_devices=NCORES)
    with ExitStack() as ctx:
        tc = ctx.enter_context(tile.TileContext(nc))
        _emit(nc, tc, ctx)
    nc.compile()
    _CACHE["nc"] = nc
    return nc


def kernel(x, patterns, length):
    """Full-input entry: x (64,200,1200) f32, patterns (400,200) f32,
    length (64,) int32 -> (graphs (64,200,200) f32, norm f32, orth f32)."""
    nc = _build()

    x16 = x.astype(np.float16).reshape(NCORES, BL, N, T)
    sap = np.floor(length.astype(np.float32) / STRIDE)
    isap2 = (1.0 / (2.0 * sap)).astype(np.float32).reshape(NCORES, BL)
    pat = np.ascontiguousarray(patterns.astype(np.float32))

    in_maps = []
    for c in range(NCORES):
        in_maps.append({
            "x16": np.ascontiguousarray(x16[c]),
            "patterns": pat,
            "invsap2": np.ascontiguousarray(
                np.broadcast_to(isap2[c][None, :], (128, BL))),
        })

    res = run_bass_kernel_spmd(nc, in_maps, core_ids=list(range(NCORES)))
    global _LAST_RESULT
    _LAST_RESULT = res
    graphs = np.concatenate([r["graphs"] for r in res.results], axis=0)
    norm = np.float32(res.results[0]["norm"].reshape(()))
    orth = np.float32(res.results[0]["orth"].reshape(()))
    return graphs, norm, orth


# revision 14
# speedup vs baseline: 1.5995x; 1.0641x over previous
"""Trainium2 Bass kernel for nn_KPFBNC (pattern-similarity graph + losses).

Computes, per batch sample b:
  graphs[b] = kp^T diag(PES[b]) kp * non_diag_scale
plus scalar losses (norm, orth) from the patterns alone.

Sharding: pure data parallelism over batch B=64 across 8 NeuronCores
(8 samples per core); patterns replicated; losses computed redundantly
on every core (taken from core 0).

Device pipeline per core:
  prep:   normalize patterns, top-3 mask, losses, PE-transpose -> patT
  per sample:
    xsq = x*x (DVE) -> column sums via ones-matmul (PE) ->
    invf = exp(-0.5*ln(fnorm2+eps)) (ACT, 4-sample batched rows) ->
    DMA broadcast of invf -> x_norm = x*invf (DVE) ->
    sim = patT^T @ x_norm (PE, fp16 in / f32 accum) ->
    ACT drains PSUM->SBUF fp16 -> windowed max/min reduces (DVE) ->
    PES -> graphs matmuls (PE) -> *non_diag_scale (DVE) -> DMA out
"""

import numpy as np
import ml_dtypes  # noqa: F401  (np float16 used; bf16 avail if needed)

import concourse.bass as bass
import concourse.bacc as bacc
import concourse.tile as tile
from concourse import mybir
from concourse.bass_utils import run_bass_kernel_spmd

# ---- problem constants (hardcoded per contract) ----
B, N, P, T = 64, 200, 400, 1200
TOPK, STRIDE = 3, 25
THRESHOLD = 0.3
EPS = 1e-9
NCORES = 8
BL = B // NCORES            # samples per core
W = T // STRIDE             # 48 windows
C_SCALE = TOPK * P / N      # 6.0 non-diag scale

F32 = mybir.dt.float32
F16 = mybir.dt.float16
AX = mybir.AxisListType
OP = mybir.AluOpType
AF = mybir.ActivationFunctionType

P_TILES = [(0, 128), (128, 128), (256, 128), (384, 16)]
N_TILES = [(0, 128), (128, 72)]
T_CHUNKS = [(0, 512), (512, 512), (1024, 176)]

BIG = 1e30


def _emit(nc, tc, ctx):
    # ---------------- IO ----------------
    x_t = nc.dram_tensor("x16", [BL, N, T], F16, kind="ExternalInput")
    pat_t = nc.dram_tensor("patterns", [P, N], F32, kind="ExternalInput")
    isap_t = nc.dram_tensor("invsap2", [128, BL], F32, kind="ExternalInput")
    g_t = nc.dram_tensor("graphs", [BL, N, N], F32, kind="ExternalOutput")
    norm_t = nc.dram_tensor("norm", [1, 1], F32, kind="ExternalOutput")
    orth_t = nc.dram_tensor("orth", [1, 1], F32, kind="ExternalOutput")
    invf_scr = nc.dram_tensor("invf_scr", [BL, T], F16, kind="Internal")

    # ---------------- inline consts ----------------
    ident_c = nc.inline_tensor(np.eye(128, dtype=np.float32), "identc")
    ones32_c = nc.inline_tensor(np.ones((128, 1), np.float32), "ones32c")
    ones16_c = nc.inline_tensor(np.ones((128, 1), np.float16), "ones16c")
    dmask_np = np.full((N, N), C_SCALE, np.float32) - np.eye(N, dtype=np.float32) * (
        C_SCALE - 1.0
    )
    dmask_c = nc.inline_tensor(dmask_np, "dmaskc")
    eyec_np = np.zeros((128, 16), np.float16)
    for _b in range(4):
        eyec_np[:, 4 * _b + _b] = 1.0
    eyec_c = nc.inline_tensor(eyec_np, "eyecolsc")

    statics = ctx.enter_context(tc.tile_pool(name="statics", bufs=1))

    ident = statics.tile([128, 128], F32, tag="ident")
    nc.sync.dma_start(out=ident, in_=ident_c.ap())
    ones32 = statics.tile([128, 1], F32, tag="ones32")
    nc.sync.dma_start(out=ones32, in_=ones32_c.ap())
    ones16 = statics.tile([128, 1], F16, tag="ones16")
    nc.sync.dma_start(out=ones16, in_=ones16_c.ap())
    eyecols = statics.tile([128, 16], F16, tag="eyecols")
    nc.sync.dma_start(out=eyecols, in_=eyec_c.ap())
    dmask = []
    for mi, (n0, nsz) in enumerate(N_TILES):
        dm = statics.tile([nsz, N], F32, tag=f"dmask{mi}")
        nc.sync.dma_start(out=dm, in_=dmask_c.ap()[n0 : n0 + nsz, :])
        dmask.append(dm)
    isap = statics.tile([128, BL], F32, tag="isap")
    nc.sync.dma_start(out=isap, in_=isap_t.ap())
    epsc = statics.tile([128, 1], F32, tag="epsc")
    nc.vector.memset(epsc, EPS)

    # =========================================================
    # PREP: patterns -> npat, top-3, losses, patT (f16 + f32)
    # =========================================================
    prep_cm = tc.tile_pool(name="prep", bufs=1)
    prep_ps_cm = tc.tile_pool(name="prep_ps", bufs=1, space="PSUM")
    loss_ps_cm = tc.tile_pool(name="loss_ps", bufs=1, space="PSUM")
    prep = prep_cm.__enter__()
    prep_ps = prep_ps_cm.__enter__()
    loss_ps = loss_ps_cm.__enter__()

    npat = []      # normalized patterns f32, per P tile
    top = []       # toppattern f32
    kp16 = []      # toppattern f16
    lenp2 = []     # per-row sum npat^2

    ps_norm = loss_ps.tile([1, 1], F32, tag="ps_norm")
    ps_orth = loss_ps.tile([1, 1], F32, tag="ps_orth")

    for pi, (p0, psz) in enumerate(P_TILES):
        pat = prep.tile([psz, N], F32, tag=f"pat{pi}")
        nc.sync.dma_start(out=pat, in_=pat_t.ap()[p0 : p0 + psz, :])

        scr = prep.tile([psz, N], F32, tag=f"scr{pi}")
        pn2 = prep.tile([psz, 1], F32, tag=f"pn2{pi}")
        nc.vector.scalar_tensor_tensor(
            out=scr, in0=pat, scalar=1.0, in1=pat, op0=OP.bypass, op1=OP.mult,
            accum_out=pn2,
        )
        slen = prep.tile([psz, 1], F32, tag=f"slen{pi}")
        nc.scalar.activation(out=slen, in_=pn2, func=AF.Sqrt,
                             bias=epsc[0:psz, 0:1], scale=1.0)
        invl = prep.tile([psz, 1], F32, tag=f"invl{pi}")
        nc.vector.reciprocal(out=invl, in_=slen)
        np_i = prep.tile([psz, N], F32, tag=f"npat{pi}")
        nc.vector.tensor_scalar_mul(out=np_i, in0=pat, scalar1=invl[:, 0:1])
        npat.append(np_i)

        # ---- top-3 by |npat| ----
        a = prep.tile([psz, N], F32, tag=f"abs{pi}")
        nc.scalar.activation(out=a, in_=np_i, func=AF.Abs)
        m1 = prep.tile([psz, 1], F32, tag=f"m1{pi}")
        nc.vector.reduce_max(out=m1, in_=a, axis=AX.X)
        e = prep.tile([psz, N], F32, tag=f"e{pi}")
        nc.vector.tensor_scalar(out=e, in0=a, scalar1=m1[:, 0:1], scalar2=None,
                                op0=OP.is_equal)
        a1 = prep.tile([psz, N], F32, tag=f"a1{pi}")
        nc.vector.scalar_tensor_tensor(
            out=a1, in0=e, scalar=-BIG, in1=a, op0=OP.mult, op1=OP.add)
        m2 = prep.tile([psz, 1], F32, tag=f"m2{pi}")
        nc.vector.reduce_max(out=m2, in_=a1, axis=AX.X)
        nc.vector.tensor_scalar(out=e, in0=a1, scalar1=m2[:, 0:1], scalar2=None,
                                op0=OP.is_equal)
        a2 = prep.tile([psz, N], F32, tag=f"a2{pi}")
        nc.vector.scalar_tensor_tensor(
            out=a2, in0=e, scalar=-BIG, in1=a1, op0=OP.mult, op1=OP.add)
        m3 = prep.tile([psz, 1], F32, tag=f"m3{pi}")
        nc.vector.reduce_max(out=m3, in_=a2, axis=AX.X)
        mask = prep.tile([psz, N], F32, tag=f"mask{pi}")
        nc.vector.tensor_scalar(out=mask, in0=a, scalar1=m3[:, 0:1], scalar2=None,
                                op0=OP.is_ge)
        t_i = statics.tile([psz, N], F32, tag=f"top{pi}")
        nc.vector.tensor_mul(t_i, mask, np_i)
        top.append(t_i)
        k_i = statics.tile([psz, N], F16, tag=f"kp16{pi}")
        nc.vector.tensor_copy(out=k_i, in_=t_i)
        kp16.append(k_i)

        # ---- norm loss pieces ----
        l2 = prep.tile([psz, 1], F32, tag=f"lenp2{pi}")
        nc.vector.scalar_tensor_tensor(
            out=scr, in0=np_i, scalar=1.0, in1=np_i, op0=OP.bypass, op1=OP.mult,
            accum_out=l2)
        lenp2.append(l2)
        q = prep.tile([psz, 1], F32, tag=f"q{pi}")
        nc.vector.scalar_tensor_tensor(
            out=scr, in0=t_i, scalar=1.0, in1=t_i, op0=OP.bypass, op1=OP.mult,
            accum_out=q)
        il2 = prep.tile([psz, 1], F32, tag=f"il2{pi}")
        nc.vector.reciprocal(out=il2, in_=l2)
        r2 = prep.tile([psz, 1], F32, tag=f"r2{pi}")
        nc.vector.tensor_mul(r2, q, il2)
        r = prep.tile([psz, 1], F32, tag=f"r{pi}")
        nc.scalar.activation(out=r, in_=r2, func=AF.Sqrt)
        tm = prep.tile([psz, 1], F32, tag=f"tm{pi}")
        nc.vector.tensor_scalar(out=tm, in0=r, scalar1=-1.0, scalar2=1.0,
                                op0=OP.mult, op1=OP.add)
        tm2 = prep.tile([psz, 1], F32, tag=f"tm2{pi}")
        nc.vector.tensor_mul(tm2, tm, tm)
        nc.tensor.matmul(ps_norm[0:1, 0:1], lhsT=tm2[:, 0:1], rhs=ones32[0:psz, 0:1],
                         start=(pi == 0), stop=(pi == len(P_TILES) - 1),
                         skip_group_check=True)

    # ---- transpose npat -> patT (two N-row tiles of (nsz, P)) ----
    patT16 = []
    patT32 = []
    for ni, (n0, nsz) in enumerate(N_TILES):
        ps_pt = prep_ps.tile([nsz, P], F32, tag=f"ps_patT{ni}")
        for pi, (p0, psz) in enumerate(P_TILES):
            nc.tensor.transpose(
                ps_pt[0:nsz, p0 : p0 + psz],
                npat[pi][:, n0 : n0 + nsz],
                ident[0:psz, 0:psz],
            )
        pt32 = statics.tile([nsz, P], F32, tag=f"patT32_{ni}")
        nc.scalar.copy(out=pt32, in_=ps_pt)
        pt16 = statics.tile([nsz, P], F16, tag=f"patT16_{ni}")
        nc.vector.tensor_copy(out=pt16, in_=ps_pt)
        patT16.append(pt16)
        patT32.append(pt32)

    # ---- orth loss ----
    for pi, (p0, psz) in enumerate(P_TILES):
        ps_o = prep_ps.tile([psz, P], F32, tag="ps_simpp")
        for ni, (n0, nsz) in enumerate(N_TILES):
            nc.tensor.matmul(ps_o, lhsT=patT32[ni][:, p0 : p0 + psz],
                             rhs=patT32[ni][:, :],
                             start=(ni == 0), stop=(ni == 1))
        ab = prep.tile([psz, P], F32, tag=f"oab{pi}")
        nc.scalar.activation(out=ab, in_=ps_o, func=AF.Abs)
        z = prep.tile([psz, P], F32, tag=f"oz{pi}")
        nc.vector.tensor_scalar(out=z, in0=ab, scalar1=-THRESHOLD,
                                scalar2=1.0 / (1.0 - THRESHOLD + 1e-5),
                                op0=OP.add, op1=OP.mult)
        rl = prep.tile([psz, P], F32, tag=f"orl{pi}")
        nc.vector.tensor_scalar_max(out=rl, in0=z, scalar1=0.0)
        oscr = prep.tile([psz, P], F32, tag=f"oscr{pi}")
        osum = prep.tile([psz, 1], F32, tag=f"osum{pi}")
        nc.vector.scalar_tensor_tensor(
            out=oscr, in0=rl, scalar=1.0, in1=rl, op0=OP.bypass, op1=OP.mult,
            accum_out=osum)
        # subtract the diagonal contribution relu((lenp2 - th)/(1-th+1e-5))^2
        zd = prep.tile([psz, 1], F32, tag=f"ozd{pi}")
        nc.vector.tensor_scalar(out=zd, in0=lenp2[pi], scalar1=-THRESHOLD,
                                scalar2=1.0 / (1.0 - THRESHOLD + 1e-5),
                                op0=OP.add, op1=OP.mult)
        nc.vector.tensor_scalar_max(out=zd, in0=zd, scalar1=0.0)
        d2 = prep.tile([psz, 1], F32, tag=f"od2{pi}")
        nc.vector.tensor_mul(d2, zd, zd)
        nc.vector.tensor_sub(osum, osum, d2)
        nc.tensor.matmul(ps_orth[0:1, 0:1], lhsT=osum[:, 0:1],
                         rhs=ones32[0:psz, 0:1],
                         start=(pi == 0), stop=(pi == len(P_TILES) - 1),
                         skip_group_check=True)

    normv = prep.tile([1, 1], F32, tag="normv")
    nc.scalar.mul(out=normv, in_=ps_norm, mul=1.0 / P)
    nc.sync.dma_start(out=norm_t.ap(), in_=normv)
    orthv = prep.tile([1, 1], F32, tag="orthv")
    nc.scalar.mul(out=orthv, in_=ps_orth, mul=1.0 / (P * P))
    nc.sync.dma_start(out=orth_t.ap(), in_=orthv)

    loss_ps_cm.__exit__(None, None, None)
    prep_ps_cm.__exit__(None, None, None)
    prep_cm.__exit__(None, None, None)

    # =========================================================
    # PER-SAMPLE PIPELINE
    # =========================================================
    xp = ctx.enter_context(tc.tile_pool(name="xp", bufs=8))
    sq = ctx.enter_context(tc.tile_pool(name="sq", bufs=8))
    rows = ctx.enter_context(tc.tile_pool(name="rows", bufs=2))
    work = ctx.enter_context(tc.tile_pool(name="work", bufs=2))
    simp = ctx.enter_context(tc.tile_pool(name="simp", bufs=4))
    fn_ps = ctx.enter_context(tc.tile_pool(name="fn_ps", bufs=2, space="PSUM"))
    sim_ps = ctx.enter_context(tc.tile_pool(name="sim_ps", bufs=5, space="PSUM"))
    g_ps = ctx.enter_context(tc.tile_pool(name="g_ps", bufs=1, space="PSUM"))

    xa = {}
    xb = {}
    xsqa = {}
    xsqb = {}

    def phase1_load(b):
        xa[b] = xp.tile([128, T], F16, tag="xa", name=f"xa{b}")
        nc.sync.dma_start(out=xa[b], in_=x_t.ap()[b, 0:128, :])
        xb[b] = xp.tile([72, T], F16, tag="xb", name=f"xb{b}")
        nc.sync.dma_start(out=xb[b], in_=x_t.ap()[b, 128:200, :])
        xsqa[b] = sq.tile([128, T], F16, tag="xsqa", name=f"xsqa{b}")
        nc.vector.tensor_mul(xsqa[b], xa[b], xa[b])
        xsqb[b] = sq.tile([72, T], F16, tag="xsqb", name=f"xsqb{b}")
        nc.gpsimd.tensor_mul(xsqb[b], xb[b], xb[b])

    def phase2_group(g):
        grp = range(4 * g, 4 * g + 4)
        lnf = rows.tile([4, T], F32, tag="lnf")
        for c, (c0, csz) in enumerate(T_CHUNKS):
            fn = fn_ps.tile([4, 512], F32, tag="fn")
            for k, b in enumerate(grp):
                for ni, (n0, nsz) in enumerate(N_TILES):
                    xs = (xsqa[b] if ni == 0 else xsqb[b])
                    nc.tensor.matmul(
                        fn[0:4, 0:csz],
                        lhsT=eyecols[0:nsz, 4 * k : 4 * k + 4],
                        rhs=xs[:, c0 : c0 + csz],
                        start=(k == 0 and ni == 0), stop=(k == 3 and ni == 1),
                        skip_group_check=True,
                    )
            nc.scalar.activation(
                out=lnf[0:4, c0 : c0 + csz],
                in_=fn[0:4, 0:csz],
                func=AF.Ln, bias=epsc[0:4, 0:1], scale=1.0,
            )
        invf = rows.tile([4, T], F16, tag="invf")
        nc.scalar.activation(out=invf[0:4, :], in_=lnf[0:4, :],
                             func=AF.Exp, scale=-0.5)
        for k, b in enumerate(grp):
            nc.sync.dma_start(out=invf_scr.ap()[b : b + 1, :],
                              in_=invf[k : k + 1, :])

    def phase3_sample(b):
        bc = work.tile([128, T], F16, tag="bc")
        nc.sync.dma_start(out=bc, in_=invf_scr.ap()[b : b + 1, :].to_broadcast([128, T]))
        xna = work.tile([128, T], F16, tag="xna")
        nc.vector.tensor_mul(xna, xa[b], bc)
        xnb = work.tile([72, T], F16, tag="xnb")
        nc.vector.tensor_mul(xnb, xb[b], bc[0:72, :])

        pes = {}
        # s-major permuted stream: PSUM col j = s*48 + w  <->  t = 25*w + s,
        # so each "group" s is 48 contiguous windows-values; pooling becomes a
        # pairwise max/min tree over contiguous f16 slabs (DVE 2x mode).
        for pi, (p0, psz) in enumerate(P_TILES):
            sim16 = simp.tile([psz, T], F16, tag="sim16")
            for c, (c0, csz) in enumerate(T_CHUNKS):
                ps_c = sim_ps.tile([psz, 512], F32, tag="sim", name=f"psc{pi}{c}")
                for ni, (n0, nsz) in enumerate(N_TILES):
                    xn = (xna if ni == 0 else xnb)
                    nc.tensor.matmul(
                        ps_c[:, 0:csz],
                        lhsT=patT16[ni][:, p0 : p0 + psz],
                        rhs=xn[:, c0 : c0 + csz],
                        start=(ni == 0), stop=(ni == 1),
                        skip_group_check=True,
                    )
                nc.scalar.activation(out=sim16[:, c0 : c0 + csz],
                                     in_=ps_c[:, 0:csz], func=AF.Copy)
            simw = sim16.rearrange("p (w s) -> p w s", s=STRIDE)
            pos = simp.tile([psz, W], F16, tag="pos")
            nc.vector.reduce_max(out=pos, in_=simw, axis=AX.X)
            neg = simp.tile([psz, W], F16, tag="neg")
            nc.vector.tensor_reduce(out=neg, in_=simw, axis=AX.X, op=OP.min)
            scr48 = simp.tile([psz, W], F16, tag="scr48")
            d = simp.tile([psz, 1], F32, tag="pesd")
            nc.vector.scalar_tensor_tensor(
                out=scr48, in0=pos, scalar=1.0, in1=neg,
                op0=OP.bypass, op1=OP.subtract, accum_out=d)
            pe = simp.tile([psz, 1], F32, tag="pes")
            nc.vector.tensor_scalar_mul(out=pe, in0=d, scalar1=isap[0:psz, b : b + 1])
            pes[pi] = pe

        skp = {}
        for pi, (p0, psz) in enumerate(P_TILES):
            sk = work.tile([psz, N], F16, tag=f"skp{pi}")
            nc.scalar.activation(out=sk, in_=top[pi], func=AF.Copy,
                                 scale=pes[pi][:, 0:1])
            skp[pi] = sk

        for mi, (n0, nsz) in enumerate(N_TILES):
            ps_g = g_ps.tile([nsz, N], F32, tag="g")
            for pi, (p0, psz) in enumerate(P_TILES):
                nc.tensor.matmul(
                    ps_g,
                    lhsT=skp[pi][:, n0 : n0 + nsz],
                    rhs=kp16[pi][:, :],
                    start=(pi == 0), stop=(pi == len(P_TILES) - 1),
                )
            gm = work.tile([nsz, N], F32, tag=f"gout{mi}")
            nc.vector.tensor_mul(gm, ps_g, dmask[mi])
            nc.sync.dma_start(out=g_t.ap()[b, n0 : n0 + nsz, :], in_=gm)

    for g in range(BL // 4):
        for b in range(4 * g, 4 * g + 4):
            phase1_load(b)
        phase2_group(g)
    for b in range(BL):
        phase3_sample(b)


_CACHE = {}
_LAST_RESULT = None


def _build():
    if "nc" in _CACHE:
        return _CACHE["nc"]
    from contextlib import ExitStack

    nc = bacc.Bacc("TRN2", target_bir_lowering=False, debug=False,
                   num_devices=NCORES)
    with ExitStack() as ctx:
        tc = ctx.enter_context(tile.TileContext(nc))
        _emit(nc, tc, ctx)
    nc.compile()
    _CACHE["nc"] = nc
    return nc


def kernel(x, patterns, length):
    """Full-input entry: x (64,200,1200) f32, patterns (400,200) f32,
    length (64,) int32 -> (graphs (64,200,200) f32, norm f32, orth f32)."""
    nc = _build()

    x16 = x.astype(np.float16).reshape(NCORES, BL, N, T)
    sap = np.floor(length.astype(np.float32) / STRIDE)
    isap2 = (1.0 / (2.0 * sap)).astype(np.float32).reshape(NCORES, BL)
    pat = np.ascontiguousarray(patterns.astype(np.float32))

    in_maps = []
    for c in range(NCORES):
        in_maps.append({
            "x16": np.ascontiguousarray(x16[c]),
            "patterns": pat,
            "invsap2": np.ascontiguousarray(
                np.broadcast_to(isap2[c][None, :], (128, BL))),
        })

    res = run_bass_kernel_spmd(nc, in_maps, core_ids=list(range(NCORES)))
    global _LAST_RESULT
    _LAST_RESULT = res
    graphs = np.concatenate([r["graphs"] for r in res.results], axis=0)
    norm = np.float32(res.results[0]["norm"].reshape(()))
    orth = np.float32(res.results[0]["orth"].reshape(()))
    return graphs, norm, orth


# revision 15
# speedup vs baseline: 1.6278x; 1.0177x over previous
"""Trainium2 Bass kernel for nn_KPFBNC (pattern-similarity graph + losses).

Computes, per batch sample b:
  graphs[b] = kp^T diag(PES[b]) kp * non_diag_scale
plus scalar losses (norm, orth) from the patterns alone.

Sharding: pure data parallelism over batch B=64 across 8 NeuronCores
(8 samples per core); patterns replicated; losses computed redundantly
on every core (taken from core 0).

Device pipeline per core:
  prep:   normalize patterns, top-3 mask, losses, PE-transpose -> patT
  per sample:
    xsq = x*x (DVE) -> column sums via ones-matmul (PE) ->
    invf = exp(-0.5*ln(fnorm2+eps)) (ACT, 4-sample batched rows) ->
    DMA broadcast of invf -> x_norm = x*invf (DVE) ->
    sim = patT^T @ x_norm (PE, fp16 in / f32 accum) ->
    ACT drains PSUM->SBUF fp16 -> windowed max/min reduces (DVE) ->
    PES -> graphs matmuls (PE) -> *non_diag_scale (DVE) -> DMA out
"""

import numpy as np
import ml_dtypes  # noqa: F401  (np float16 used; bf16 avail if needed)

import concourse.bass as bass
import concourse.bacc as bacc
import concourse.tile as tile
from concourse import mybir
from concourse.bass_utils import run_bass_kernel_spmd

# ---- problem constants (hardcoded per contract) ----
B, N, P, T = 64, 200, 400, 1200
TOPK, STRIDE = 3, 25
THRESHOLD = 0.3
EPS = 1e-9
NCORES = 8
BL = B // NCORES            # samples per core
W = T // STRIDE             # 48 windows
C_SCALE = TOPK * P / N      # 6.0 non-diag scale

F32 = mybir.dt.float32
F16 = mybir.dt.float16
AX = mybir.AxisListType
OP = mybir.AluOpType
AF = mybir.ActivationFunctionType

P_TILES = [(0, 128), (128, 128), (256, 128), (384, 16)]
N_TILES = [(0, 128), (128, 72)]
T_CHUNKS = [(0, 512), (512, 512), (1024, 176)]

BIG = 1e30


def _emit(nc, tc, ctx):
    # ---------------- IO ----------------
    x_t = nc.dram_tensor("x16", [BL, N, T], F16, kind="ExternalInput")
    pat_t = nc.dram_tensor("patterns", [P, N], F32, kind="ExternalInput")
    isap_t = nc.dram_tensor("invsap2", [128, BL], F32, kind="ExternalInput")
    g_t = nc.dram_tensor("graphs", [BL, N, N], F32, kind="ExternalOutput")
    norm_t = nc.dram_tensor("norm", [1, 1], F32, kind="ExternalOutput")
    orth_t = nc.dram_tensor("orth", [1, 1], F32, kind="ExternalOutput")
    invf_scr = nc.dram_tensor("invf_scr", [BL, T], F16, kind="Internal")

    # ---------------- inline consts ----------------
    ident_c = nc.inline_tensor(np.eye(128, dtype=np.float32), "identc")
    ones32_c = nc.inline_tensor(np.ones((128, 1), np.float32), "ones32c")
    ones16_c = nc.inline_tensor(np.ones((128, 1), np.float16), "ones16c")
    dmask_np = np.full((N, N), C_SCALE, np.float32) - np.eye(N, dtype=np.float32) * (
        C_SCALE - 1.0
    )
    dmask_c = nc.inline_tensor(dmask_np, "dmaskc")
    eyec_np = np.zeros((128, 16), np.float16)
    for _b in range(4):
        eyec_np[:, 4 * _b + _b] = 1.0
    eyec_c = nc.inline_tensor(eyec_np, "eyecolsc")

    statics = ctx.enter_context(tc.tile_pool(name="statics", bufs=1))

    ident = statics.tile([128, 128], F32, tag="ident")
    nc.sync.dma_start(out=ident, in_=ident_c.ap())
    ones32 = statics.tile([128, 1], F32, tag="ones32")
    nc.sync.dma_start(out=ones32, in_=ones32_c.ap())
    ones16 = statics.tile([128, 1], F16, tag="ones16")
    nc.sync.dma_start(out=ones16, in_=ones16_c.ap())
    eyecols = statics.tile([128, 16], F16, tag="eyecols")
    nc.sync.dma_start(out=eyecols, in_=eyec_c.ap())
    dmask = []
    for mi, (n0, nsz) in enumerate(N_TILES):
        dm = statics.tile([nsz, N], F32, tag=f"dmask{mi}")
        nc.sync.dma_start(out=dm, in_=dmask_c.ap()[n0 : n0 + nsz, :])
        dmask.append(dm)
    isap = statics.tile([128, BL], F32, tag="isap")
    nc.sync.dma_start(out=isap, in_=isap_t.ap())
    epsc = statics.tile([128, 1], F32, tag="epsc")
    nc.vector.memset(epsc, EPS)

    # =========================================================
    # PREP: patterns -> npat, top-3, losses, patT (f16 + f32)
    # =========================================================
    prep_cm = tc.tile_pool(name="prep", bufs=1)
    prep_ps_cm = tc.tile_pool(name="prep_ps", bufs=1, space="PSUM")
    loss_ps_cm = tc.tile_pool(name="loss_ps", bufs=1, space="PSUM")
    prep = prep_cm.__enter__()
    prep_ps = prep_ps_cm.__enter__()
    loss_ps = loss_ps_cm.__enter__()

    npat = []      # normalized patterns f32, per P tile
    top = []       # toppattern f32
    kp16 = []      # toppattern f16
    lenp2 = []     # per-row sum npat^2

    ps_norm = loss_ps.tile([1, 1], F32, tag="ps_norm")
    ps_orth = loss_ps.tile([1, 1], F32, tag="ps_orth")

    for pi, (p0, psz) in enumerate(P_TILES):
        pat = prep.tile([psz, N], F32, tag=f"pat{pi}")
        nc.sync.dma_start(out=pat, in_=pat_t.ap()[p0 : p0 + psz, :])

        scr = prep.tile([psz, N], F32, tag=f"scr{pi}")
        pn2 = prep.tile([psz, 1], F32, tag=f"pn2{pi}")
        nc.vector.scalar_tensor_tensor(
            out=scr, in0=pat, scalar=1.0, in1=pat, op0=OP.bypass, op1=OP.mult,
            accum_out=pn2,
        )
        slen = prep.tile([psz, 1], F32, tag=f"slen{pi}")
        nc.scalar.activation(out=slen, in_=pn2, func=AF.Sqrt,
                             bias=epsc[0:psz, 0:1], scale=1.0)
        invl = prep.tile([psz, 1], F32, tag=f"invl{pi}")
        nc.vector.reciprocal(out=invl, in_=slen)
        np_i = prep.tile([psz, N], F32, tag=f"npat{pi}")
        nc.vector.tensor_scalar_mul(out=np_i, in0=pat, scalar1=invl[:, 0:1])
        npat.append(np_i)

        # ---- top-3 by |npat| ----
        a = prep.tile([psz, N], F32, tag=f"abs{pi}")
        nc.scalar.activation(out=a, in_=np_i, func=AF.Abs)
        m1 = prep.tile([psz, 1], F32, tag=f"m1{pi}")
        nc.vector.reduce_max(out=m1, in_=a, axis=AX.X)
        e = prep.tile([psz, N], F32, tag=f"e{pi}")
        nc.vector.tensor_scalar(out=e, in0=a, scalar1=m1[:, 0:1], scalar2=None,
                                op0=OP.is_equal)
        a1 = prep.tile([psz, N], F32, tag=f"a1{pi}")
        nc.vector.scalar_tensor_tensor(
            out=a1, in0=e, scalar=-BIG, in1=a, op0=OP.mult, op1=OP.add)
        m2 = prep.tile([psz, 1], F32, tag=f"m2{pi}")
        nc.vector.reduce_max(out=m2, in_=a1, axis=AX.X)
        nc.vector.tensor_scalar(out=e, in0=a1, scalar1=m2[:, 0:1], scalar2=None,
                                op0=OP.is_equal)
        a2 = prep.tile([psz, N], F32, tag=f"a2{pi}")
        nc.vector.scalar_tensor_tensor(
            out=a2, in0=e, scalar=-BIG, in1=a1, op0=OP.mult, op1=OP.add)
        m3 = prep.tile([psz, 1], F32, tag=f"m3{pi}")
        nc.vector.reduce_max(out=m3, in_=a2, axis=AX.X)
        mask = prep.tile([psz, N], F32, tag=f"mask{pi}")
        nc.vector.tensor_scalar(out=mask, in0=a, scalar1=m3[:, 0:1], scalar2=None,
                                op0=OP.is_ge)
        t_i = statics.tile([psz, N], F32, tag=f"top{pi}")
        nc.vector.tensor_mul(t_i, mask, np_i)
        top.append(t_i)
        k_i = statics.tile([psz, N], F16, tag=f"kp16{pi}")
        nc.vector.tensor_copy(out=k_i, in_=t_i)
        kp16.append(k_i)

        # ---- norm loss pieces ----
        l2 = prep.tile([psz, 1], F32, tag=f"lenp2{pi}")
        nc.vector.scalar_tensor_tensor(
            out=scr, in0=np_i, scalar=1.0, in1=np_i, op0=OP.bypass, op1=OP.mult,
            accum_out=l2)
        lenp2.append(l2)
        q = prep.tile([psz, 1], F32, tag=f"q{pi}")
        nc.vector.scalar_tensor_tensor(
            out=scr, in0=t_i, scalar=1.0, in1=t_i, op0=OP.bypass, op1=OP.mult,
            accum_out=q)
        il2 = prep.tile([psz, 1], F32, tag=f"il2{pi}")
        nc.vector.reciprocal(out=il2, in_=l2)
        r2 = prep.tile([psz, 1], F32, tag=f"r2{pi}")
        nc.vector.tensor_mul(r2, q, il2)
        r = prep.tile([psz, 1], F32, tag=f"r{pi}")
        nc.scalar.activation(out=r, in_=r2, func=AF.Sqrt)
        tm = prep.tile([psz, 1], F32, tag=f"tm{pi}")
        nc.vector.tensor_scalar(out=tm, in0=r, scalar1=-1.0, scalar2=1.0,
                                op0=OP.mult, op1=OP.add)
        tm2 = prep.tile([psz, 1], F32, tag=f"tm2{pi}")
        nc.vector.tensor_mul(tm2, tm, tm)
        nc.tensor.matmul(ps_norm[0:1, 0:1], lhsT=tm2[:, 0:1], rhs=ones32[0:psz, 0:1],
                         start=(pi == 0), stop=(pi == len(P_TILES) - 1),
                         skip_group_check=True)

    # ---- transpose npat -> patT (two N-row tiles of (nsz, P)) ----
    patT16 = []
    patT32 = []
    for ni, (n0, nsz) in enumerate(N_TILES):
        ps_pt = prep_ps.tile([nsz, P], F32, tag=f"ps_patT{ni}")
        for pi, (p0, psz) in enumerate(P_TILES):
            nc.tensor.transpose(
                ps_pt[0:nsz, p0 : p0 + psz],
                npat[pi][:, n0 : n0 + nsz],
                ident[0:psz, 0:psz],
            )
        pt32 = statics.tile([nsz, P], F32, tag=f"patT32_{ni}")
        nc.scalar.copy(out=pt32, in_=ps_pt)
        pt16 = statics.tile([nsz, P], F16, tag=f"patT16_{ni}")
        nc.vector.tensor_copy(out=pt16, in_=ps_pt)
        patT16.append(pt16)
        patT32.append(pt32)

    # ---- orth loss ----
    for pi, (p0, psz) in enumerate(P_TILES):
        ps_o = prep_ps.tile([psz, P], F32, tag="ps_simpp")
        for ni, (n0, nsz) in enumerate(N_TILES):
            nc.tensor.matmul(ps_o, lhsT=patT32[ni][:, p0 : p0 + psz],
                             rhs=patT32[ni][:, :],
                             start=(ni == 0), stop=(ni == 1))
        ab = prep.tile([psz, P], F32, tag=f"oab{pi}")
        nc.scalar.activation(out=ab, in_=ps_o, func=AF.Abs)
        z = prep.tile([psz, P], F32, tag=f"oz{pi}")
        nc.vector.tensor_scalar(out=z, in0=ab, scalar1=-THRESHOLD,
                                scalar2=1.0 / (1.0 - THRESHOLD + 1e-5),
                                op0=OP.add, op1=OP.mult)
        rl = prep.tile([psz, P], F32, tag=f"orl{pi}")
        nc.vector.tensor_scalar_max(out=rl, in0=z, scalar1=0.0)
        oscr = prep.tile([psz, P], F32, tag=f"oscr{pi}")
        osum = prep.tile([psz, 1], F32, tag=f"osum{pi}")
        nc.vector.scalar_tensor_tensor(
            out=oscr, in0=rl, scalar=1.0, in1=rl, op0=OP.bypass, op1=OP.mult,
            accum_out=osum)
        # subtract the diagonal contribution relu((lenp2 - th)/(1-th+1e-5))^2
        zd = prep.tile([psz, 1], F32, tag=f"ozd{pi}")
        nc.vector.tensor_scalar(out=zd, in0=lenp2[pi], scalar1=-THRESHOLD,
                                scalar2=1.0 / (1.0 - THRESHOLD + 1e-5),
                                op0=OP.add, op1=OP.mult)
        nc.vector.tensor_scalar_max(out=zd, in0=zd, scalar1=0.0)
        d2 = prep.tile([psz, 1], F32, tag=f"od2{pi}")
        nc.vector.tensor_mul(d2, zd, zd)
        nc.vector.tensor_sub(osum, osum, d2)
        nc.tensor.matmul(ps_orth[0:1, 0:1], lhsT=osum[:, 0:1],
                         rhs=ones32[0:psz, 0:1],
                         start=(pi == 0), stop=(pi == len(P_TILES) - 1),
                         skip_group_check=True)

    normv = prep.tile([1, 1], F32, tag="normv")
    nc.scalar.mul(out=normv, in_=ps_norm, mul=1.0 / P)
    nc.sync.dma_start(out=norm_t.ap(), in_=normv)
    orthv = prep.tile([1, 1], F32, tag="orthv")
    nc.scalar.mul(out=orthv, in_=ps_orth, mul=1.0 / (P * P))
    nc.sync.dma_start(out=orth_t.ap(), in_=orthv)

    loss_ps_cm.__exit__(None, None, None)
    prep_ps_cm.__exit__(None, None, None)
    prep_cm.__exit__(None, None, None)

    # =========================================================
    # PER-SAMPLE PIPELINE
    # =========================================================
    xp = ctx.enter_context(tc.tile_pool(name="xp", bufs=8))
    sq = ctx.enter_context(tc.tile_pool(name="sq", bufs=8))
    rows = ctx.enter_context(tc.tile_pool(name="rows", bufs=2))
    work = ctx.enter_context(tc.tile_pool(name="work", bufs=3))
    simp = ctx.enter_context(tc.tile_pool(name="simp", bufs=4))
    fn_ps = ctx.enter_context(tc.tile_pool(name="fn_ps", bufs=2, space="PSUM"))
    sim_ps = ctx.enter_context(tc.tile_pool(name="sim_ps", bufs=5, space="PSUM"))
    g_ps = ctx.enter_context(tc.tile_pool(name="g_ps", bufs=1, space="PSUM"))

    xa = {}
    xb = {}
    xsqa = {}
    xsqb = {}

    def phase1_load(b):
        xa[b] = xp.tile([128, T], F16, tag="xa", name=f"xa{b}")
        nc.sync.dma_start(out=xa[b], in_=x_t.ap()[b, 0:128, :])
        xb[b] = xp.tile([72, T], F16, tag="xb", name=f"xb{b}")
        nc.sync.dma_start(out=xb[b], in_=x_t.ap()[b, 128:200, :])
        xsqa[b] = sq.tile([128, T], F16, tag="xsqa", name=f"xsqa{b}")
        nc.vector.tensor_mul(xsqa[b], xa[b], xa[b])
        xsqb[b] = sq.tile([72, T], F16, tag="xsqb", name=f"xsqb{b}")
        nc.gpsimd.tensor_mul(xsqb[b], xb[b], xb[b])

    def phase2_group(g):
        grp = range(4 * g, 4 * g + 4)
        lnf = rows.tile([4, T], F32, tag="lnf")
        for c, (c0, csz) in enumerate(T_CHUNKS):
            fn = fn_ps.tile([4, 512], F32, tag="fn")
            for k, b in enumerate(grp):
                for ni, (n0, nsz) in enumerate(N_TILES):
                    xs = (xsqa[b] if ni == 0 else xsqb[b])
                    nc.tensor.matmul(
                        fn[0:4, 0:csz],
                        lhsT=eyecols[0:nsz, 4 * k : 4 * k + 4],
                        rhs=xs[:, c0 : c0 + csz],
                        start=(k == 0 and ni == 0), stop=(k == 3 and ni == 1),
                        skip_group_check=True,
                    )
            nc.scalar.activation(
                out=lnf[0:4, c0 : c0 + csz],
                in_=fn[0:4, 0:csz],
                func=AF.Ln, bias=epsc[0:4, 0:1], scale=1.0,
            )
        invf = rows.tile([4, T], F16, tag="invf")
        nc.scalar.activation(out=invf[0:4, :], in_=lnf[0:4, :],
                             func=AF.Exp, scale=-0.5)
        for k, b in enumerate(grp):
            nc.sync.dma_start(out=invf_scr.ap()[b : b + 1, :],
                              in_=invf[k : k + 1, :])

    def phase3_sample(b):
        bc = work.tile([128, T], F16, tag="bc")
        nc.sync.dma_start(out=bc, in_=invf_scr.ap()[b : b + 1, :].to_broadcast([128, T]))
        xna = work.tile([128, T], F16, tag="xna")
        nc.vector.tensor_mul(xna, xa[b], bc)
        xnb = work.tile([72, T], F16, tag="xnb")
        nc.vector.tensor_mul(xnb, xb[b], bc[0:72, :])

        pes = {}
        # s-major permuted stream: PSUM col j = s*48 + w  <->  t = 25*w + s,
        # so each "group" s is 48 contiguous windows-values; pooling becomes a
        # pairwise max/min tree over contiguous f16 slabs (DVE 2x mode).
        for pi, (p0, psz) in enumerate(P_TILES):
            sim16 = simp.tile([psz, T], F16, tag="sim16")
            for c, (c0, csz) in enumerate(T_CHUNKS):
                ps_c = sim_ps.tile([psz, 512], F32, tag="sim", name=f"psc{pi}{c}")
                for ni, (n0, nsz) in enumerate(N_TILES):
                    xn = (xna if ni == 0 else xnb)
                    nc.tensor.matmul(
                        ps_c[:, 0:csz],
                        lhsT=patT16[ni][:, p0 : p0 + psz],
                        rhs=xn[:, c0 : c0 + csz],
                        start=(ni == 0), stop=(ni == 1),
                        skip_group_check=True,
                    )
                nc.scalar.activation(out=sim16[:, c0 : c0 + csz],
                                     in_=ps_c[:, 0:csz], func=AF.Copy)
            simw = sim16.rearrange("p (w s) -> p w s", s=STRIDE)
            pos = simp.tile([psz, W], F16, tag="pos")
            nc.vector.reduce_max(out=pos, in_=simw, axis=AX.X)
            neg = simp.tile([psz, W], F16, tag="neg")
            nc.vector.tensor_reduce(out=neg, in_=simw, axis=AX.X, op=OP.min)
            scr48 = simp.tile([psz, W], F16, tag="scr48")
            d = simp.tile([psz, 1], F32, tag="pesd")
            nc.vector.scalar_tensor_tensor(
                out=scr48, in0=pos, scalar=1.0, in1=neg,
                op0=OP.bypass, op1=OP.subtract, accum_out=d)
            pe = simp.tile([psz, 1], F32, tag="pes")
            nc.vector.tensor_scalar_mul(out=pe, in0=d, scalar1=isap[0:psz, b : b + 1])
            pes[pi] = pe

        skp = {}
        for pi, (p0, psz) in enumerate(P_TILES):
            sk = work.tile([psz, N], F16, tag=f"skp{pi}")
            nc.scalar.activation(out=sk, in_=top[pi], func=AF.Copy,
                                 scale=pes[pi][:, 0:1])
            skp[pi] = sk

        for mi, (n0, nsz) in enumerate(N_TILES):
            ps_g = g_ps.tile([nsz, N], F32, tag="g")
            for pi, (p0, psz) in enumerate(P_TILES):
                nc.tensor.matmul(
                    ps_g,
                    lhsT=skp[pi][:, n0 : n0 + nsz],
                    rhs=kp16[pi][:, :],
                    start=(pi == 0), stop=(pi == len(P_TILES) - 1),
                )
            gm = work.tile([nsz, N], F32, tag=f"gout{mi}")
            nc.vector.tensor_mul(gm, ps_g, dmask[mi])
            nc.sync.dma_start(out=g_t.ap()[b, n0 : n0 + nsz, :], in_=gm)

    for g in range(BL // 4):
        for b in range(4 * g, 4 * g + 4):
            phase1_load(b)
        phase2_group(g)
    for b in range(BL):
        phase3_sample(b)


_CACHE = {}
_LAST_RESULT = None


def _build():
    if "nc" in _CACHE:
        return _CACHE["nc"]
    from contextlib import ExitStack

    nc = bacc.Bacc("TRN2", target_bir_lowering=False, debug=False,
                   num_devices=NCORES)
    with ExitStack() as ctx:
        tc = ctx.enter_context(tile.TileContext(nc))
        _emit(nc, tc, ctx)
    nc.compile()
    _CACHE["nc"] = nc
    return nc


def kernel(x, patterns, length):
    """Full-input entry: x (64,200,1200) f32, patterns (400,200) f32,
    length (64,) int32 -> (graphs (64,200,200) f32, norm f32, orth f32)."""
    nc = _build()

    x16 = x.astype(np.float16).reshape(NCORES, BL, N, T)
    sap = np.floor(length.astype(np.float32) / STRIDE)
    isap2 = (1.0 / (2.0 * sap)).astype(np.float32).reshape(NCORES, BL)
    pat = np.ascontiguousarray(patterns.astype(np.float32))

    in_maps = []
    for c in range(NCORES):
        in_maps.append({
            "x16": np.ascontiguousarray(x16[c]),
            "patterns": pat,
            "invsap2": np.ascontiguousarray(
                np.broadcast_to(isap2[c][None, :], (128, BL))),
        })

    res = run_bass_kernel_spmd(nc, in_maps, core_ids=list(range(NCORES)))
    global _LAST_RESULT
    _LAST_RESULT = res
    graphs = np.concatenate([r["graphs"] for r in res.results], axis=0)
    norm = np.float32(res.results[0]["norm"].reshape(()))
    orth = np.float32(res.results[0]["orth"].reshape(()))
    return graphs, norm, orth
